# revision 26
# baseline (speedup 1.0000x reference)
"""CCA (cross-covariance / channel) attention kernel for Trainium2, 8 NeuronCores.

Math (per batch element b, all derived from the reference nn.Module):
    qkv = x @ W_qkv ; per head h: q,k,v in [N, 64] layouts
    channel attention: attn_h = softmax_d( (q_hat^T k_hat) * temp_h ),
    with q_hat = q / ||q||_col (L2 over N), out = attn @ v^T, y = out^T @ W_proj + b.

Key factorization used here (N=4096 >> C=512):
    S = x^T x                      [512,512]   (shared across heads)
    qk_h = Wq_h^T S Wk_h,  |q_c|^2 = diag(Wq_h^T S Wq_h)  (via T = S @ Wqk)
    M_h = attn_h^T Wp_h            [64,512]
    P   = sum_h Wv_h M_h           [512,512]
    y   = x @ P + b                 (big matmul, uses host-pretransposed x^T)

The whole S->T->qk->softmax path is scale-invariant (the cosine
normalization cancels any uniform scale on S), so it runs in fp8
DoubleRow end to end: x is pre-scaled by 1/8 on the host so S/64 falls
out of PSUM in fp8 range with no on-chip rescale.  The y = x @ P matmul
feeds the output directly, so it stays bf16.

Norms use a 64-row ones matmul so n^2 lands broadcast across 64
partitions; r = sqrt(1/n^2) then runs on wide tiles (DVE reciprocal +
ACT sqrt) and rk is just a strided view of the result - no
single-partition Ln/Exp rows.

Data-parallel over B=8 across the 8 cores; no collectives.
"""

import os
import sys
import numpy as np

for _p in ("/opt/trn_rl_repo",):
    if _p not in sys.path and os.path.isdir(_p):
        sys.path.insert(0, _p)

import ml_dtypes  # noqa: E402
from contextlib import ExitStack  # noqa: E402

import functools  # noqa: E402

import concourse.bass as bass  # noqa: E402
import concourse.bacc as bacc  # noqa: E402
import concourse.hw_specs as hw_specs  # noqa: E402


@functools.cache
def _patched_act_tables(arch):
    # Keep Ln/Exp only in natural_log_exp_and_others so the table-load pass
    # resolves both to ONE set (a single ~1.3us ACT_TABLE_LOAD per kernel).
    base = hw_specs.get_activation_tables(arch)
    out = {}
    for name, fns in base.items():
        fns = set(fns)
        if name != "natural_log_exp_and_others":
            fns -= {mybir.ActivationFunctionType.Ln, mybir.ActivationFunctionType.Exp}
        out[name] = fns
    return out


bacc.get_activation_tables = _patched_act_tables
import concourse.tile as tile  # noqa: E402
from concourse import mybir  # noqa: E402
from concourse.bass_utils import run_bass_kernel_spmd  # noqa: E402
from concourse.tile_rust import add_dep_helper  # noqa: E402

B, N, C = 8, 4096, 512
NH, HD = 8, 64
NT = N // 128  # 32 n-tiles
KC = C // 128  # 4 contraction chunks of 128
GP = NH // 2   # 4 head-pairs for the P phase
F32 = mybir.dt.float32
BF16 = mybir.dt.bfloat16
FP8 = mybir.dt.float8e4
AF = mybir.ActivationFunctionType
ALU = mybir.AluOpType
DR = mybir.MatmulPerfMode.DoubleRow
BF16_NP = ml_dtypes.bfloat16
FP8_NP = ml_dtypes.float8_e4m3


def _build_kernel_body(ctx: ExitStack, tc: tile.TileContext, io: dict):
    nc = tc.nc
    x_nat, x_tr, wqk8, wqkb, wv2, wp2, bpr, temp, y = (
        io["x_nat"], io["x_tr"], io["wqk8"], io["wqkb"], io["wv2"],
        io["wp2"], io["bpr"], io["temp"], io["y"],
    )

    persist = ctx.enter_context(tc.tile_pool(name="persist", bufs=1))
    ypool = ctx.enter_context(tc.tile_pool(name="ypool", bufs=6))
    psum = ctx.enter_context(tc.tile_pool(name="psum", bufs=6, space="PSUM"))
    psum_g = ctx.enter_context(tc.tile_pool(name="psum_g", bufs=1, space="PSUM"))

    # ---- PE prewarm (emitted first so the tensor queue ramps the clock
    # while the first x chunk is still in flight) ---------------------------
    scr_sb = persist.tile([128, C], BF16)
    nc.vector.memset(scr_sb, 1.0)
    for i in range(10):
        kp = psum.tile([128, C], F32, tag="work_ps", name=f"prewarm{i}")
        nc.tensor.matmul(
            kp[:, 0:64], scr_sb[:, 0:128], scr_sb[:, 0:64], start=True, stop=True
        )

    # ---- loads -------------------------------------------------------------
    # x (fp8, pre-scaled by 1/8, feeds only S) is host-pre-tiled to
    # [128, NT, C]; streamed in 4 chunks spread across all three DMA queues
    # (2 HWDGE + gpsimd SWDGE) since a single queue runs at ~90 GB/s.
    CHUNK_TILES = [8, 8, 8, 8]
    CHUNK_ENG = [nc.sync, nc.scalar, nc.gpsimd, nc.sync]
    NCHUNK = len(CHUNK_TILES)
    x_chunks = []
    x_dmas = []
    t0 = 0
    for c, ntc in enumerate(CHUNK_TILES):
        xc = persist.tile([128, ntc, C], FP8, tag=f"x_chunk{c}")
        x_dmas.append(CHUNK_ENG[c].dma_start(out=xc, in_=x_nat[:, t0:t0 + ntc, :]))
        x_chunks.append(xc)
        t0 += ntc
    wqk8_sb = persist.tile([128, KC, 2 * C], FP8)
    nc.scalar.dma_start(out=wqk8_sb, in_=wqk8[:])
    ident128 = persist.tile([128, 128], FP8)
    nc.scalar.dma_start(out=ident128, in_=io["ident"][:])
    wqkb_sb = persist.tile([128, KC, 2 * C], BF16)
    nc.gpsimd.dma_start(out=wqkb_sb, in_=wqkb[:])
    wv2_sb = persist.tile([128, GP, C], BF16)  # [(two,d), pair, ci]
    nc.gpsimd.dma_start(out=wv2_sb, in_=wv2[:])
    wp_sb = persist.tile([64, NH, C], BF16)  # [c, (h, e)]
    nc.gpsimd.dma_start(out=wp_sb, in_=wp2[:])
    bias_sb = persist.tile([128, C], F32)
    nc.gpsimd.dma_start(
        out=bias_sb,
        in_=bass.AP(tensor=bpr[:].tensor, offset=bpr[:].offset, ap=[[0, 128], [1, C]]),
    )
    temp_b = persist.tile([64, NH], F32)
    nc.gpsimd.dma_start(
        out=temp_b,
        in_=bass.AP(tensor=temp[:].tensor, offset=temp[:].offset,
                    ap=[[0, 64], [1, NH]]),
    )
    ones_bc = persist.tile([128, HD], BF16)  # 64-wide ones: norms broadcast
    nc.vector.memset(ones_bc, 1.0)
    # xT (bf16, feeds only the y phase) streams behind the x chunks across
    # all three queues.  Lands well before the y phase needs it.
    xt_sb = persist.tile([128, KC, N], BF16)
    xt_view = x_tr[:].rearrange("(k p) n -> p k n", p=128)
    XT_ENG = [nc.sync, nc.scalar, nc.gpsimd]
    for g in range(6):
        xd = XT_ENG[g % 3].dma_start(
            out=xt_sb[:, :, g * 683:min(N, (g + 1) * 683)],
            in_=xt_view[:, :, g * 683:min(N, (g + 1) * 683)],
        )
        add_dep_helper(xd.ins, x_dmas[-1].ins,
                       reason="xT load deferred behind S inputs")

    # ACT table warmup (Exp for softmax; emitted early so the table load
    # happens during the DMA ramp).
    warm_sb = persist.tile([1, 2], F32)
    nc.vector.memset(warm_sb, 1.0)
    nc.scalar.activation(warm_sb[:, 1:2], warm_sb[:, 1:2], AF.Exp)
    nc.scalar.activation(warm_sb[:, 0:1], warm_sb[:, 0:1], AF.Ln)

    # small dependency-paced PE keepalive for the softmax/norms lulls: keep()
    # waits on the chain tensor, dense() adds real PE density behind it.
    _keep_n = [0]

    def keep(dep):
        kp = psum.tile([1, 2], F32, tag="work_ps", name=f"keep{_keep_n[0]}")
        _keep_n[0] += 1
        nc.tensor.matmul(kp[:, 0:1], dep, dep, start=True, stop=True)

    def dense(n):
        for _ in range(n):
            kp = psum.tile([128, C], F32, tag="work_ps", name=f"dense{_keep_n[0]}")
            _keep_n[0] += 1
            nc.tensor.matmul(
                kp[:, 0:256], wqk8_sb[:, 0, 0:128], wqk8_sb[:, 0, 0:256],
                start=True, stop=True,
            )

    # ---- S = (x/8)^T (x/8) = S_true/64  [C, C], fp8 DoubleRow -------------
    # S is symmetric: compute only the upper-triangle block-rows (rhs width
    # shrinks 512/384/256/128) and mirror the 6 lower blocks via PE
    # transposes.  Chunk-outer loop so accumulation starts when the first x
    # chunk arrives; the last chunk goes kc-sequential so each s8 copy
    # overlaps the next bank's remaining matmuls.
    s8_sb = persist.tile([128, KC, C], FP8)
    s_ps = [
        psum.tile([128, C - 128 * kc], F32, tag="work_ps", name=f"s_ps{kc}")
        for kc in range(KC)
    ]
    for c in range(NCHUNK - 1):
        for kc in range(KC):
            for tp in range(CHUNK_TILES[c] // 2):
                nc.tensor.matmul(
                    s_ps[kc],
                    x_chunks[c][:, 2 * tp:2 * tp + 2, kc * 128:(kc + 1) * 128],
                    x_chunks[c][:, 2 * tp:2 * tp + 2, kc * 128:],
                    perf_mode=DR,
                    start=(c == 0 and tp == 0),
                    stop=False,
                )
    for kc in range(KC):
        cl = NCHUNK - 1
        for tp in range(CHUNK_TILES[cl] // 2):
            nc.tensor.matmul(
                s_ps[kc],
                x_chunks[cl][:, 2 * tp:2 * tp + 2, kc * 128:(kc + 1) * 128],
                x_chunks[cl][:, 2 * tp:2 * tp + 2, kc * 128:],
                perf_mode=DR,
                start=False,
                stop=(tp == CHUNK_TILES[cl] // 2 - 1),
            )
        # S -> fp8 (no rescale needed; host pre-scaled x); overlaps the next
        # bank's chunk-3 matmuls.
        if kc % 2 == 0:
            nc.scalar.copy(s8_sb[:, kc, kc * 128:], s_ps[kc])
        else:
            nc.vector.tensor_copy(s8_sb[:, kc, kc * 128:], s_ps[kc])
    # mirror the lower-triangle blocks: s8[j, i] = s8[i, j]^T
    nmir = 0
    for i in range(KC):
        for j in range(i + 1, KC):
            # fp8 transpose writes with an output element step of 2
            mir_ps = psum.tile([128, 128, 2], FP8, tag="work_ps", name=f"mir{i}{j}")
            mir_v = mir_ps[:, :, 0]
            nc.tensor.transpose(
                mir_v, s8_sb[:, i, j * 128:(j + 1) * 128], ident128
            )
            if nmir % 2 == 0:
                nc.scalar.copy(s8_sb[:, j, i * 128:(i + 1) * 128], mir_v)
            else:
                nc.vector.tensor_copy(s8_sb[:, j, i * 128:(i + 1) * 128], mir_v)
            nmir += 1

    # ---- T = S8 @ Wqk8 [C, 2C] in fp8 DoubleRow ---------------------------
    # Only the k-half of T feeds the qk matmuls (fp8 copies); pn = Wqk*T
    # (both halves, read straight from PSUM) feeds the norms reduction and
    # runs on the otherwise-idle gpsimd engine.
    t8_sb = persist.tile([128, KC, C], FP8)  # k-half of T only
    pn_sb = persist.tile([128, KC, 2 * C], BF16)
    for ti in range(KC):
        for half in range(2):
            t_ps = psum.tile([128, C], F32, tag="work_ps")
            for jp in range(2):
                nc.tensor.matmul(
                    t_ps,
                    s8_sb[:, 2 * jp:2 * jp + 2, ti * 128:(ti + 1) * 128],
                    wqk8_sb[:, 2 * jp:2 * jp + 2, half * C:(half + 1) * C],
                    perf_mode=DR,
                    start=(jp == 0),
                    stop=(jp == 1),
                )
            # pn on DVE (gpsimd can't read PSUM), t8 k-copies on ACT
            nc.vector.tensor_mul(
                pn_sb[:, ti, half * C:(half + 1) * C],
                wqkb_sb[:, ti, half * C:(half + 1) * C],
                t_ps,
            )
            # one strided copy pulls this half's 4 per-head k-col blocks
            # ([hh*128+64 : hh*128+128]) into t8 packed as (h, d)
            ksrc = t_ps.rearrange("p (hh s) -> p hh s", s=128)[:, :, HD:]
            kdst = t8_sb[:, ti, half * 4 * HD:(half + 1) * 4 * HD].rearrange(
                "p (hh d) -> p hh d", d=HD
            )
            nc.scalar.copy(kdst, ksrc)

    # ---- qk_h = Wq8_h^T T8_k(h)  [64, 64] per head, fp8 DoubleRow ---------
    # (head-outer, pair-inner: PSUM accumulation groups are bank-scoped)
    qk_ps = psum_g.tile([64, NH, HD], F32)
    for h in range(NH):
        for jp in range(2):
            nc.tensor.matmul(
                qk_ps[:, h, :],
                wqk8_sb[:, 2 * jp:2 * jp + 2, h * 128:h * 128 + HD],
                t8_sb[:, 2 * jp:2 * jp + 2, h * HD:(h + 1) * HD],
                perf_mode=DR,
                start=(jp == 0),
                stop=(jp == 1),
            )

    # ---- norms: n2 broadcast via 64-row ones matmuls, r = sqrt(1/n2) ------
    # nrm_ps[half][c, w] = n2[w] for every c<64; rk is then just a strided
    # view of r_bc and rq comes from 8 tiny PE transposes of row 0.
    pn2_sb = persist.tile([128, 2, 2 * C], BF16)
    nc.vector.tensor_add(pn2_sb[:, 0, :], pn_sb[:, 0, :], pn_sb[:, 1, :])
    nc.vector.tensor_add(pn2_sb[:, 1, :], pn_sb[:, 2, :], pn_sb[:, 3, :])
    nrm_ps = [
        psum.tile([64, C], F32, tag="work_ps", name=f"nrm_ps{half}")
        for half in range(2)
    ]
    for half in range(2):
        for ti in range(2):
            nc.tensor.matmul(
                nrm_ps[half],
                ones_bc,
                pn2_sb[:, ti, half * C:(half + 1) * C],
                start=(ti == 0),
                stop=(ti == 1),
            )
    dense(3)  # unpaced: keeps the PE clock up through the ln/exp window
    lnr_bc = persist.tile([64, 2 * C], F32)
    for half in range(2):
        nc.scalar.activation(
            lnr_bc[:, half * C:(half + 1) * C], nrm_ps[half], AF.Ln
        )
    r_bc = persist.tile([64, 2 * C], BF16)  # [c, (h, {q64|k64})] broadcast
    nc.scalar.activation(r_bc, lnr_bc, AF.Exp, scale=-0.5)
    keep(r_bc[0:1, 0:1])
    dense(1)

    # rq: r_bc's q-slices transposed to the partition dim, * temperature
    ident1 = persist.tile([1, 1], BF16)
    nc.vector.memset(ident1, 1.0)
    tr_ps = psum.tile([64, 2 * NH], BF16, tag="work_ps")
    for h in range(NH):
        nc.tensor.transpose(
            tr_ps[:, 2 * h:2 * h + 1], r_bc[0:1, h * 128:h * 128 + HD], ident1
        )
    rq_sb = persist.tile([64, NH], F32)
    tr_view = tr_ps.rearrange("p (h two) -> p h two", two=2)[:, :, 0]
    nc.vector.tensor_mul(rq_sb, tr_view, temp_b)  # fold temperature into rq
    rk_view = r_bc.rearrange("p (h s) -> p h s", s=128)[:, :, HD:]  # [64,NH,HD]
    dense(2)  # unpaced: covers the logits DVE window

    # ---- softmax (all heads fused) -> M -> P (head-pair packed) -----------
    # |logits| <= max(temperature) so exp() is safe without max-subtraction.
    #
    # Engines can't shift partitions, so the pair packing happens inside the
    # M matmuls: attn is written into a zero-padded stationary layout
    # attn_pad[:, j, two, two*64:(two+1)*64] and the two matmuls of pair j
    # accumulate M_even into PSUM partitions 0:64 and M_odd into 64:128 of
    # one [128, C] tile.  P then contracts 128 rows (2 heads) per matmul.
    lg = persist.tile([64, NH, HD], F32)
    ex = persist.tile([64, NH, HD], F32)
    ssum = persist.tile([64, NH], F32)
    attn_pad = persist.tile([64, GP, 2, 2 * HD], BF16)
    nc.vector.memset(attn_pad, 0.0)
    m2_sb = persist.tile([128, GP, C], BF16)  # [(two,d), pair, e]
    p_ps = [
        psum.tile([128, C], F32, tag="work_ps", name=f"p_ps{t}") for t in range(KC)
    ]

    nc.vector.tensor_mul(lg, qk_ps, rk_view)
    nc.vector.tensor_mul(
        lg, lg, rq_sb[:, :, None].broadcast_to([64, NH, HD])
    )
    keep(lg[0:1, NH - 1, 0:1])
    dense(1)
    nc.scalar.activation(ex, lg, AF.Exp)
    keep(ex[0:1, NH - 1, 0:1])
    dense(1)
    nc.vector.tensor_reduce(
        ssum[:, :, None], ex, axis=mybir.AxisListType.X, op=ALU.add
    )
    nc.vector.reciprocal(ssum, ssum)
    # evens -> attn_pad[:, :, 0, 0:64], odds -> attn_pad[:, :, 1, 64:128]
    ex_v = ex.rearrange("p (j two) d -> p j two d", two=2)
    ss_v = ssum.rearrange("p (j two) -> p j two", two=2)
    nc.vector.tensor_mul(
        attn_pad[:, :, 0, 0:HD],
        ex_v[:, :, 0, :],
        ss_v[:, :, 0, None].broadcast_to([64, GP, HD]),
    )
    nc.vector.tensor_mul(
        attn_pad[:, :, 1, HD:2 * HD],
        ex_v[:, :, 1, :],
        ss_v[:, :, 1, None].broadcast_to([64, GP, HD]),
    )

    def emit_p(j):  # accumulate head-pair j into all four P row-blocks
        for t in range(KC):
            nc.tensor.matmul(
                p_ps[t],
                wv2_sb[:, j, t * 128:(t + 1) * 128],
                m2_sb[:, j, :],
                start=(j == 0),
                stop=(j == GP - 1),
            )

    # M/P software pipeline: P(pair j-1) runs while m2[j] is being filled
    for j in range(GP):
        m2_ps = psum.tile([128, C], F32, tag="work_ps")
        for two in range(2):
            nc.tensor.matmul(
                m2_ps,
                attn_pad[:, j, two, :],
                wp_sb[:, 2 * j + two, :],
                start=(two == 0),
                stop=(two == 1),
            )
        if j % 2 == 0:
            nc.scalar.copy(m2_sb[:, j, :], m2_ps)
        else:
            nc.vector.tensor_copy(m2_sb[:, j, :], m2_ps)
        if j > 0:
            emit_p(j - 1)
    emit_p(GP - 1)

    p_sb = persist.tile([128, KC, C], BF16)
    for t in range(KC):
        if t % 2 == 0:
            nc.scalar.copy(p_sb[:, t, :], p_ps[t])
        else:
            nc.vector.tensor_copy(p_sb[:, t, :], p_ps[t])

    # ---- y = x @ P + b ----------------------------------------------------
    # paired-tile stores: one DMA per two n-tiles, round-robin on all queues.
    y_paired = y[:].rearrange("(t p) c -> p t c", p=128)
    ST_ENG = [nc.sync, nc.scalar, nc.gpsimd]
    ybuf = None
    for t in range(NT):
        y_ps = psum.tile([128, C], F32, tag="work_ps")
        for kc in range(KC):
            nc.tensor.matmul(
                y_ps,
                xt_sb[:, kc, t * 128:(t + 1) * 128],
                p_sb[:, kc, :],
                start=(kc == 0),
                stop=(kc == KC - 1),
            )
        if t >= NT - 4:
            # single-tile stores at the end so the drain finishes sooner
            ybuf = ypool.tile([128, 1, C], BF16)
            nc.vector.tensor_add(ybuf[:, 0, :], y_ps, bias_sb)
            ST_ENG[t % 3].dma_start(out=y_paired[:, t:t + 1, :], in_=ybuf)
        else:
            if t % 2 == 0:
                ybuf = ypool.tile([128, 2, C], BF16)
            nc.vector.tensor_add(ybuf[:, t % 2, :], y_ps, bias_sb)
            if t % 2 == 1:
                eng = ST_ENG[(t // 2) % 3]
                eng.dma_start(out=y_paired[:, t - 1:t + 1, :], in_=ybuf)


def build_nc():
    nc = bacc.Bacc("TRN2", target_bir_lowering=False, debug=False, num_devices=B)
    io = {}
    io["x_nat"] = nc.dram_tensor("x_nat", [128, NT, C], FP8, kind="ExternalInput")
    io["x_tr"] = nc.dram_tensor("x_tr", [C, N], BF16, kind="ExternalInput")
    io["wqk8"] = nc.dram_tensor("wqk8", [128, KC, 2 * C], FP8, kind="ExternalInput")
    io["ident"] = nc.dram_tensor("ident", [128, 128], FP8, kind="ExternalInput")
    io["wqkb"] = nc.dram_tensor("wqkb", [128, KC, 2 * C], BF16, kind="ExternalInput")
    io["wv2"] = nc.dram_tensor("wv2", [128, GP, C], BF16, kind="ExternalInput")
    io["wp2"] = nc.dram_tensor("wp2", [64, NH, C], BF16, kind="ExternalInput")
    io["bpr"] = nc.dram_tensor("bpr", [C], F32, kind="ExternalInput")
    io["temp"] = nc.dram_tensor("temp", [NH], F32, kind="ExternalInput")
    io["y"] = nc.dram_tensor("y", [N, C], BF16, kind="ExternalOutput")
    with tile.TileContext(nc) as tc:
        with ExitStack() as ctx:
            _build_kernel_body(ctx, tc, io)
    nc.compile()
    return nc


_NC_CACHE = None


def _get_nc():
    global _NC_CACHE
    if _NC_CACHE is None:
        _NC_CACHE = build_nc()
    return _NC_CACHE


def prep_host_inputs(x, W_qkv, temperature, W_proj, b_proj):
    """Host-side preprocessing shared by all cores. Returns per-core in_maps."""
    x = np.asarray(x, dtype=np.float32)
    W_qkv = np.asarray(W_qkv, dtype=np.float32)
    temperature = np.asarray(temperature, dtype=np.float32).reshape(NH)
    W_proj = np.asarray(W_proj, dtype=np.float32)
    b_proj = np.asarray(b_proj, dtype=np.float32)

    Wq = W_qkv[:, 0:C].reshape(C, NH, HD)
    Wk = W_qkv[:, C:2 * C].reshape(C, NH, HD)
    wqk_perm = np.concatenate([Wq, Wk], axis=2).reshape(C, 2 * C)  # [(ci),(h)(qk c)]
    wqk_tiled = np.ascontiguousarray(
        wqk_perm.reshape(KC, 128, 2 * C).transpose(1, 0, 2)
    )  # [p, kc, 2C]
    Wv = W_qkv[:, 2 * C:3 * C]  # [ci, (h d)]
    # [(two,d), pair, ci] so head-pairs stack on the partition dim for P
    wv2 = np.ascontiguousarray(
        Wv.T.reshape(GP, 2, HD, C).transpose(1, 2, 0, 3).reshape(128, GP, C)
    )
    wp2 = np.ascontiguousarray(
        W_proj.reshape(NH, HD, C).transpose(1, 0, 2)
    )  # [c, h, e]

    wqk8 = wqk_tiled.astype(FP8_NP)
    wqkb = wqk_tiled.astype(BF16_NP)
    ident = np.eye(128, dtype=np.float32).astype(FP8_NP)
    wv2_bf = wv2.astype(BF16_NP)
    wp2_bf = wp2.astype(BF16_NP)

    in_maps = []
    for b in range(B):
        xb = x[b]
        in_maps.append({
            "x_nat": np.ascontiguousarray(
                (xb * 0.125).reshape(NT, 128, C).transpose(1, 0, 2)
            ).astype(FP8_NP),
            "x_tr": np.ascontiguousarray(xb.T).astype(BF16_NP),
            "wqk8": wqk8,
            "ident": ident,
            "wqkb": wqkb,
            "wv2": wv2_bf,
            "wp2": wp2_bf,
            "bpr": b_proj,
            "temp": temperature,
        })
    return in_maps


def kernel(**inputs):
    x = inputs["x"]
    in_maps = prep_host_inputs(
        x, inputs["W_qkv"], inputs["temperature"], inputs["W_proj"], inputs["b_proj"]
    )
    nc = _get_nc()
    res = run_bass_kernel_spmd(nc, in_maps, list(range(B)))
    y = np.stack([np.asarray(res.results[i]["y"]) for i in range(B)], axis=0)
    return y.astype(np.float32)


if __name__ == "__main__":
    # smoke test with random data
    rng = np.random.default_rng(0)
    ins = {
        "x": rng.standard_normal((B, N, C), dtype=np.float32),
        "x_out": rng.standard_normal((B, N, C), dtype=np.float32),
        "W_qkv": (rng.standard_normal((C, 3 * C), dtype=np.float32) / np.sqrt(C)),
        "temperature": np.ones((NH, 1, 1), np.float32),
        "W_proj": (rng.standard_normal((C, C), dtype=np.float32) / np.sqrt(C)),
        "b_proj": rng.standard_normal((C,), dtype=np.float32) * 0.01,
        "H": 64,
        "W": 64,
    }
    out = kernel(**ins)
    print("out", out.shape, out.dtype, float(np.abs(out).max()))


# revision 27
# speedup vs baseline: 1.0066x; 1.0066x over previous
"""CCA (cross-covariance / channel) attention kernel for Trainium2, 8 NeuronCores.

Math (per batch element b, all derived from the reference nn.Module):
    qkv = x @ W_qkv ; per head h: q,k,v in [N, 64] layouts
    channel attention: attn_h = softmax_d( (q_hat^T k_hat) * temp_h ),
    with q_hat = q / ||q||_col (L2 over N), out = attn @ v^T, y = out^T @ W_proj + b.

Key factorization used here (N=4096 >> C=512):
    S = x^T x                      [512,512]   (shared across heads)
    qk_h = Wq_h^T S Wk_h,  |q_c|^2 = diag(Wq_h^T S Wq_h)  (via T = S @ Wqk)
    M_h = attn_h^T Wp_h            [64,512]
    P   = sum_h Wv_h M_h           [512,512]
    y   = x @ P + b                 (big matmul, uses host-pretransposed x^T)

The whole S->T->qk->softmax path is scale-invariant (the cosine
normalization cancels any uniform scale on S), so it runs in fp8
DoubleRow end to end: x is pre-scaled by 1/8 on the host so S/64 falls
out of PSUM in fp8 range with no on-chip rescale.  The y = x @ P matmul
feeds the output directly, so it stays bf16.

Norms use a 64-row ones matmul so n^2 lands broadcast across 64
partitions; r = sqrt(1/n^2) then runs on wide tiles (DVE reciprocal +
ACT sqrt) and rk is just a strided view of the result - no
single-partition Ln/Exp rows.

Data-parallel over B=8 across the 8 cores; no collectives.
"""

import os
import sys
import numpy as np

for _p in ("/opt/trn_rl_repo",):
    if _p not in sys.path and os.path.isdir(_p):
        sys.path.insert(0, _p)

import ml_dtypes  # noqa: E402
from contextlib import ExitStack  # noqa: E402

import functools  # noqa: E402

import concourse.bass as bass  # noqa: E402
import concourse.bacc as bacc  # noqa: E402
import concourse.hw_specs as hw_specs  # noqa: E402


@functools.cache
def _patched_act_tables(arch):
    # Keep Ln/Exp only in natural_log_exp_and_others so the table-load pass
    # resolves both to ONE set (a single ~1.3us ACT_TABLE_LOAD per kernel).
    base = hw_specs.get_activation_tables(arch)
    out = {}
    for name, fns in base.items():
        fns = set(fns)
        if name != "natural_log_exp_and_others":
            fns -= {mybir.ActivationFunctionType.Ln, mybir.ActivationFunctionType.Exp}
        out[name] = fns
    return out


bacc.get_activation_tables = _patched_act_tables
import concourse.tile as tile  # noqa: E402
from concourse import mybir  # noqa: E402
from concourse.bass_utils import run_bass_kernel_spmd  # noqa: E402
from concourse.tile_rust import add_dep_helper  # noqa: E402

B, N, C = 8, 4096, 512
NH, HD = 8, 64
NT = N // 128  # 32 n-tiles
KC = C // 128  # 4 contraction chunks of 128
GP = NH // 2   # 4 head-pairs for the P phase
F32 = mybir.dt.float32
BF16 = mybir.dt.bfloat16
FP8 = mybir.dt.float8e4
AF = mybir.ActivationFunctionType
ALU = mybir.AluOpType
DR = mybir.MatmulPerfMode.DoubleRow
BF16_NP = ml_dtypes.bfloat16
FP8_NP = ml_dtypes.float8_e4m3


def _build_kernel_body(ctx: ExitStack, tc: tile.TileContext, io: dict):
    nc = tc.nc
    x_nat, x_tr, wqk8, wqkb, wv2, wp2, bpr, temp, y = (
        io["x_nat"], io["x_tr"], io["wqk8"], io["wqkb"], io["wv2"],
        io["wp2"], io["bpr"], io["temp"], io["y"],
    )

    persist = ctx.enter_context(tc.tile_pool(name="persist", bufs=1))
    ypool = ctx.enter_context(tc.tile_pool(name="ypool", bufs=6))
    psum = ctx.enter_context(tc.tile_pool(name="psum", bufs=6, space="PSUM"))
    psum_g = ctx.enter_context(tc.tile_pool(name="psum_g", bufs=1, space="PSUM"))

    # ---- PE prewarm (emitted first so the tensor queue ramps the clock
    # while the first x chunk is still in flight) ---------------------------
    scr_sb = persist.tile([128, C], BF16)
    nc.vector.memset(scr_sb, 1.0)
    for i in range(4):
        kp = psum.tile([128, C], F32, tag="work_ps", name=f"prewarm{i}")
        nc.tensor.matmul(kp, scr_sb[:, 0:128], scr_sb, start=True, stop=True)

    # ---- loads -------------------------------------------------------------
    # x (fp8, pre-scaled by 1/8, feeds only S) is host-pre-tiled to
    # [128, NT, C]; streamed in 4 chunks spread across all three DMA queues
    # (2 HWDGE + gpsimd SWDGE) since a single queue runs at ~90 GB/s.
    CHUNK_TILES = [4, 8, 8, 12]
    CHUNK_ENG = [nc.sync, nc.scalar, nc.gpsimd, nc.sync]
    NCHUNK = len(CHUNK_TILES)
    x_chunks = []
    x_dmas = []
    t0 = 0
    for c, ntc in enumerate(CHUNK_TILES):
        xc = persist.tile([128, ntc, C], FP8, tag=f"x_chunk{c}")
        x_dmas.append(CHUNK_ENG[c].dma_start(out=xc, in_=x_nat[:, t0:t0 + ntc, :]))
        x_chunks.append(xc)
        t0 += ntc
    wqk8_sb = persist.tile([128, KC, 2 * C], FP8)
    nc.scalar.dma_start(out=wqk8_sb, in_=wqk8[:])
    ident128 = persist.tile([128, 128], FP8)
    nc.scalar.dma_start(out=ident128, in_=io["ident"][:])
    wqkb_sb = persist.tile([128, KC, 2 * C], BF16)
    nc.gpsimd.dma_start(out=wqkb_sb, in_=wqkb[:])
    wv2_sb = persist.tile([128, GP, C], BF16)  # [(two,d), pair, ci]
    nc.gpsimd.dma_start(out=wv2_sb, in_=wv2[:])
    wp_sb = persist.tile([64, NH, C], BF16)  # [c, (h, e)]
    nc.gpsimd.dma_start(out=wp_sb, in_=wp2[:])
    bias_sb = persist.tile([128, C], F32)
    nc.gpsimd.dma_start(
        out=bias_sb,
        in_=bass.AP(tensor=bpr[:].tensor, offset=bpr[:].offset, ap=[[0, 128], [1, C]]),
    )
    temp_b = persist.tile([64, NH], F32)
    nc.gpsimd.dma_start(
        out=temp_b,
        in_=bass.AP(tensor=temp[:].tensor, offset=temp[:].offset,
                    ap=[[0, 64], [1, NH]]),
    )
    ones_bc = persist.tile([128, HD], BF16)  # 64-wide ones: norms broadcast
    nc.vector.memset(ones_bc, 1.0)
    # xT (bf16, feeds only the y phase) streams behind the x chunks across
    # all three queues.  Lands well before the y phase needs it.
    xt_sb = persist.tile([128, KC, N], BF16)
    xt_view = x_tr[:].rearrange("(k p) n -> p k n", p=128)
    XT_ENG = [nc.sync, nc.scalar, nc.gpsimd]
    for g in range(6):
        xd = XT_ENG[g % 3].dma_start(
            out=xt_sb[:, :, g * 683:min(N, (g + 1) * 683)],
            in_=xt_view[:, :, g * 683:min(N, (g + 1) * 683)],
        )
        add_dep_helper(xd.ins, x_dmas[-1].ins,
                       reason="xT load deferred behind S inputs")

    # ACT table warmup (Exp for softmax; emitted early so the table load
    # happens during the DMA ramp).
    warm_sb = persist.tile([1, 2], F32)
    nc.vector.memset(warm_sb, 1.0)
    nc.scalar.activation(warm_sb[:, 1:2], warm_sb[:, 1:2], AF.Exp)
    nc.scalar.activation(warm_sb[:, 0:1], warm_sb[:, 0:1], AF.Ln)

    # small dependency-paced PE keepalive for the softmax/norms lulls: keep()
    # waits on the chain tensor, dense() adds real PE density behind it.
    _keep_n = [0]

    def keep(dep):
        kp = psum.tile([1, 2], F32, tag="work_ps", name=f"keep{_keep_n[0]}")
        _keep_n[0] += 1
        nc.tensor.matmul(kp[:, 0:1], dep, dep, start=True, stop=True)

    def dense(n):
        for _ in range(n):
            kp = psum.tile([128, C], F32, tag="work_ps", name=f"dense{_keep_n[0]}")
            _keep_n[0] += 1
            nc.tensor.matmul(
                kp, wqk8_sb[:, 0, 0:128], wqk8_sb[:, 0, 0:C], start=True, stop=True
            )

    # ---- S = (x/8)^T (x/8) = S_true/64  [C, C], fp8 DoubleRow -------------
    # S is symmetric: compute only the upper-triangle block-rows (rhs width
    # shrinks 512/384/256/128) and mirror the 6 lower blocks via PE
    # transposes.  Chunk-outer loop so accumulation starts when the first x
    # chunk arrives; the last chunk goes kc-sequential so each s8 copy
    # overlaps the next bank's remaining matmuls.
    s8_sb = persist.tile([128, KC, C], FP8)
    s_ps = [
        psum.tile([128, C - 128 * kc], F32, tag="work_ps", name=f"s_ps{kc}")
        for kc in range(KC)
    ]
    for c in range(NCHUNK - 1):
        for kc in range(KC):
            for tp in range(CHUNK_TILES[c] // 2):
                nc.tensor.matmul(
                    s_ps[kc],
                    x_chunks[c][:, 2 * tp:2 * tp + 2, kc * 128:(kc + 1) * 128],
                    x_chunks[c][:, 2 * tp:2 * tp + 2, kc * 128:],
                    perf_mode=DR,
                    start=(c == 0 and tp == 0),
                    stop=False,
                )
    for kc in range(KC):
        cl = NCHUNK - 1
        for tp in range(CHUNK_TILES[cl] // 2):
            nc.tensor.matmul(
                s_ps[kc],
                x_chunks[cl][:, 2 * tp:2 * tp + 2, kc * 128:(kc + 1) * 128],
                x_chunks[cl][:, 2 * tp:2 * tp + 2, kc * 128:],
                perf_mode=DR,
                start=False,
                stop=(tp == CHUNK_TILES[cl] // 2 - 1),
            )
        # S -> fp8 (no rescale needed; host pre-scaled x); overlaps the next
        # bank's chunk-3 matmuls.
        if kc % 2 == 0:
            nc.scalar.copy(s8_sb[:, kc, kc * 128:], s_ps[kc])
        else:
            nc.vector.tensor_copy(s8_sb[:, kc, kc * 128:], s_ps[kc])
    # mirror the lower-triangle blocks: s8[j, i] = s8[i, j]^T
    nmir = 0
    for i in range(KC):
        for j in range(i + 1, KC):
            # fp8 transpose writes with an output element step of 2
            mir_ps = psum.tile([128, 128, 2], FP8, tag="work_ps", name=f"mir{i}{j}")
            mir_v = mir_ps[:, :, 0]
            nc.tensor.transpose(
                mir_v, s8_sb[:, i, j * 128:(j + 1) * 128], ident128
            )
            if nmir % 2 == 0:
                nc.scalar.copy(s8_sb[:, j, i * 128:(i + 1) * 128], mir_v)
            else:
                nc.vector.tensor_copy(s8_sb[:, j, i * 128:(i + 1) * 128], mir_v)
            nmir += 1

    # ---- T = S8 @ Wqk8 [C, 2C] in fp8 DoubleRow ---------------------------
    # Only the k-half of T feeds the qk matmuls (fp8 copies); pn = Wqk*T
    # (both halves, read straight from PSUM) feeds the norms reduction and
    # runs on the otherwise-idle gpsimd engine.
    t8_sb = persist.tile([128, KC, C], FP8)  # k-half of T only
    pn_sb = persist.tile([128, KC, 2 * C], BF16)
    for ti in range(KC):
        for half in range(2):
            t_ps = psum.tile([128, C], F32, tag="work_ps")
            for jp in range(2):
                nc.tensor.matmul(
                    t_ps,
                    s8_sb[:, 2 * jp:2 * jp + 2, ti * 128:(ti + 1) * 128],
                    wqk8_sb[:, 2 * jp:2 * jp + 2, half * C:(half + 1) * C],
                    perf_mode=DR,
                    start=(jp == 0),
                    stop=(jp == 1),
                )
            # pn on DVE (gpsimd can't read PSUM), t8 k-copies on ACT
            nc.vector.tensor_mul(
                pn_sb[:, ti, half * C:(half + 1) * C],
                wqkb_sb[:, ti, half * C:(half + 1) * C],
                t_ps,
            )
            # one strided copy pulls this half's 4 per-head k-col blocks
            # ([hh*128+64 : hh*128+128]) into t8 packed as (h, d)
            ksrc = t_ps.rearrange("p (hh s) -> p hh s", s=128)[:, :, HD:]
            kdst = t8_sb[:, ti, half * 4 * HD:(half + 1) * 4 * HD].rearrange(
                "p (hh d) -> p hh d", d=HD
            )
            nc.scalar.copy(kdst, ksrc)

    # ---- qk_h = Wq8_h^T T8_k(h)  [64, 64] per head, fp8 DoubleRow ---------
    # (head-outer, pair-inner: PSUM accumulation groups are bank-scoped)
    qk_ps = psum_g.tile([64, NH, HD], F32)
    for h in range(NH):
        for jp in range(2):
            nc.tensor.matmul(
                qk_ps[:, h, :],
                wqk8_sb[:, 2 * jp:2 * jp + 2, h * 128:h * 128 + HD],
                t8_sb[:, 2 * jp:2 * jp + 2, h * HD:(h + 1) * HD],
                perf_mode=DR,
                start=(jp == 0),
                stop=(jp == 1),
            )

    # ---- norms: n2 broadcast via 64-row ones matmuls, r = sqrt(1/n2) ------
    # nrm_ps[half][c, w] = n2[w] for every c<64; rk is then just a strided
    # view of r_bc and rq comes from 8 tiny PE transposes of row 0.
    nrm_ps = [
        psum.tile([64, C], F32, tag="work_ps", name=f"nrm_ps{half}")
        for half in range(2)
    ]
    for half in range(2):
        for ti in range(KC):
            nc.tensor.matmul(
                nrm_ps[half],
                ones_bc,
                pn_sb[:, ti, half * C:(half + 1) * C],
                start=(ti == 0),
                stop=(ti == KC - 1),
            )
    lnr_bc = persist.tile([64, 2 * C], F32)
    for half in range(2):
        nc.scalar.activation(
            lnr_bc[:, half * C:(half + 1) * C], nrm_ps[half], AF.Ln
        )
    r_bc = persist.tile([64, 2 * C], BF16)  # [c, (h, {q64|k64})] broadcast
    nc.scalar.activation(r_bc, lnr_bc, AF.Exp, scale=-0.5)
    keep(r_bc[0:1, 0:1])
    dense(2)

    # rq: r_bc's q-slices transposed to the partition dim, * temperature
    ident1 = persist.tile([1, 1], BF16)
    nc.vector.memset(ident1, 1.0)
    tr_ps = psum.tile([64, 2 * NH], BF16, tag="work_ps")
    for h in range(NH):
        nc.tensor.transpose(
            tr_ps[:, 2 * h:2 * h + 1], r_bc[0:1, h * 128:h * 128 + HD], ident1
        )
    rq_sb = persist.tile([64, NH], F32)
    tr_view = tr_ps.rearrange("p (h two) -> p h two", two=2)[:, :, 0]
    nc.vector.tensor_mul(rq_sb, tr_view, temp_b)  # fold temperature into rq
    rk_view = r_bc.rearrange("p (h s) -> p h s", s=128)[:, :, HD:]  # [64,NH,HD]

    # ---- softmax (all heads fused) -> M -> P (head-pair packed) -----------
    # |logits| <= max(temperature) so exp() is safe without max-subtraction.
    #
    # Engines can't shift partitions, so the pair packing happens inside the
    # M matmuls: attn is written into a zero-padded stationary layout
    # attn_pad[:, j, two, two*64:(two+1)*64] and the two matmuls of pair j
    # accumulate M_even into PSUM partitions 0:64 and M_odd into 64:128 of
    # one [128, C] tile.  P then contracts 128 rows (2 heads) per matmul.
    lg = persist.tile([64, NH, HD], F32)
    ex = persist.tile([64, NH, HD], F32)
    ssum = persist.tile([64, NH], F32)
    attn_pad = persist.tile([64, GP, 2, 2 * HD], BF16)
    nc.vector.memset(attn_pad, 0.0)
    m2_sb = persist.tile([128, GP, C], BF16)  # [(two,d), pair, e]
    p_ps = [
        psum.tile([128, C], F32, tag="work_ps", name=f"p_ps{t}") for t in range(KC)
    ]

    nc.vector.tensor_mul(lg, qk_ps, rk_view)
    nc.vector.tensor_mul(
        lg, lg, rq_sb[:, :, None].broadcast_to([64, NH, HD])
    )
    keep(lg[0:1, NH - 1, 0:1])
    dense(1)
    nc.scalar.activation(ex, lg, AF.Exp)
    keep(ex[0:1, NH - 1, 0:1])
    dense(1)
    nc.vector.tensor_reduce(
        ssum[:, :, None], ex, axis=mybir.AxisListType.X, op=ALU.add
    )
    nc.vector.reciprocal(ssum, ssum)
    # evens -> attn_pad[:, :, 0, 0:64], odds -> attn_pad[:, :, 1, 64:128]
    ex_v = ex.rearrange("p (j two) d -> p j two d", two=2)
    ss_v = ssum.rearrange("p (j two) -> p j two", two=2)
    nc.vector.tensor_mul(
        attn_pad[:, :, 0, 0:HD],
        ex_v[:, :, 0, :],
        ss_v[:, :, 0, None].broadcast_to([64, GP, HD]),
    )
    nc.vector.tensor_mul(
        attn_pad[:, :, 1, HD:2 * HD],
        ex_v[:, :, 1, :],
        ss_v[:, :, 1, None].broadcast_to([64, GP, HD]),
    )

    def emit_p(j):  # accumulate head-pair j into all four P row-blocks
        for t in range(KC):
            nc.tensor.matmul(
                p_ps[t],
                wv2_sb[:, j, t * 128:(t + 1) * 128],
                m2_sb[:, j, :],
                start=(j == 0),
                stop=(j == GP - 1),
            )

    # M/P software pipeline: P(pair j-1) runs while m2[j] is being filled
    for j in range(GP):
        m2_ps = psum.tile([128, C], F32, tag="work_ps")
        for two in range(2):
            nc.tensor.matmul(
                m2_ps,
                attn_pad[:, j, two, :],
                wp_sb[:, 2 * j + two, :],
                start=(two == 0),
                stop=(two == 1),
            )
        if j % 2 == 0:
            nc.scalar.copy(m2_sb[:, j, :], m2_ps)
        else:
            nc.vector.tensor_copy(m2_sb[:, j, :], m2_ps)
        if j > 0:
            emit_p(j - 1)
    emit_p(GP - 1)

    p_sb = persist.tile([128, KC, C], BF16)
    for t in range(KC):
        if t % 2 == 0:
            nc.scalar.copy(p_sb[:, t, :], p_ps[t])
        else:
            nc.vector.tensor_copy(p_sb[:, t, :], p_ps[t])

    # ---- y = x @ P + b ----------------------------------------------------
    # paired-tile stores: one DMA per two n-tiles, round-robin on all queues.
    y_paired = y[:].rearrange("(t p) c -> p t c", p=128)
    ST_ENG = [nc.sync, nc.scalar, nc.gpsimd]
    ybuf = None
    for t in range(NT):
        y_ps = psum.tile([128, C], F32, tag="work_ps")
        for kc in range(KC):
            nc.tensor.matmul(
                y_ps,
                xt_sb[:, kc, t * 128:(t + 1) * 128],
                p_sb[:, kc, :],
                start=(kc == 0),
                stop=(kc == KC - 1),
            )
        if t >= NT - 4:
            # single-tile stores at the end so the drain finishes sooner
            ybuf = ypool.tile([128, 1, C], BF16)
            nc.vector.tensor_add(ybuf[:, 0, :], y_ps, bias_sb)
            ST_ENG[t % 3].dma_start(out=y_paired[:, t:t + 1, :], in_=ybuf)
        else:
            if t % 2 == 0:
                ybuf = ypool.tile([128, 2, C], BF16)
            nc.vector.tensor_add(ybuf[:, t % 2, :], y_ps, bias_sb)
            if t % 2 == 1:
                eng = ST_ENG[(t // 2) % 3]
                eng.dma_start(out=y_paired[:, t - 1:t + 1, :], in_=ybuf)


def build_nc():
    nc = bacc.Bacc("TRN2", target_bir_lowering=False, debug=False, num_devices=B)
    io = {}
    io["x_nat"] = nc.dram_tensor("x_nat", [128, NT, C], FP8, kind="ExternalInput")
    io["x_tr"] = nc.dram_tensor("x_tr", [C, N], BF16, kind="ExternalInput")
    io["wqk8"] = nc.dram_tensor("wqk8", [128, KC, 2 * C], FP8, kind="ExternalInput")
    io["ident"] = nc.dram_tensor("ident", [128, 128], FP8, kind="ExternalInput")
    io["wqkb"] = nc.dram_tensor("wqkb", [128, KC, 2 * C], BF16, kind="ExternalInput")
    io["wv2"] = nc.dram_tensor("wv2", [128, GP, C], BF16, kind="ExternalInput")
    io["wp2"] = nc.dram_tensor("wp2", [64, NH, C], BF16, kind="ExternalInput")
    io["bpr"] = nc.dram_tensor("bpr", [C], F32, kind="ExternalInput")
    io["temp"] = nc.dram_tensor("temp", [NH], F32, kind="ExternalInput")
    io["y"] = nc.dram_tensor("y", [N, C], BF16, kind="ExternalOutput")
    with tile.TileContext(nc) as tc:
        with ExitStack() as ctx:
            _build_kernel_body(ctx, tc, io)
    nc.compile()
    return nc


_NC_CACHE = None


def _get_nc():
    global _NC_CACHE
    if _NC_CACHE is None:
        _NC_CACHE = build_nc()
    return _NC_CACHE


def prep_host_inputs(x, W_qkv, temperature, W_proj, b_proj):
    """Host-side preprocessing shared by all cores. Returns per-core in_maps."""
    x = np.asarray(x, dtype=np.float32)
    W_qkv = np.asarray(W_qkv, dtype=np.float32)
    temperature = np.asarray(temperature, dtype=np.float32).reshape(NH)
    W_proj = np.asarray(W_proj, dtype=np.float32)
    b_proj = np.asarray(b_proj, dtype=np.float32)

    Wq = W_qkv[:, 0:C].reshape(C, NH, HD)
    Wk = W_qkv[:, C:2 * C].reshape(C, NH, HD)
    wqk_perm = np.concatenate([Wq, Wk], axis=2).reshape(C, 2 * C)  # [(ci),(h)(qk c)]
    wqk_tiled = np.ascontiguousarray(
        wqk_perm.reshape(KC, 128, 2 * C).transpose(1, 0, 2)
    )  # [p, kc, 2C]
    Wv = W_qkv[:, 2 * C:3 * C]  # [ci, (h d)]
    # [(two,d), pair, ci] so head-pairs stack on the partition dim for P
    wv2 = np.ascontiguousarray(
        Wv.T.reshape(GP, 2, HD, C).transpose(1, 2, 0, 3).reshape(128, GP, C)
    )
    wp2 = np.ascontiguousarray(
        W_proj.reshape(NH, HD, C).transpose(1, 0, 2)
    )  # [c, h, e]

    wqk8 = wqk_tiled.astype(FP8_NP)
    wqkb = wqk_tiled.astype(BF16_NP)
    ident = np.eye(128, dtype=np.float32).astype(FP8_NP)
    wv2_bf = wv2.astype(BF16_NP)
    wp2_bf = wp2.astype(BF16_NP)

    in_maps = []
    for b in range(B):
        xb = x[b]
        in_maps.append({
            "x_nat": np.ascontiguousarray(
                (xb * 0.125).reshape(NT, 128, C).transpose(1, 0, 2)
            ).astype(FP8_NP),
            "x_tr": np.ascontiguousarray(xb.T).astype(BF16_NP),
            "wqk8": wqk8,
            "ident": ident,
            "wqkb": wqkb,
            "wv2": wv2_bf,
            "wp2": wp2_bf,
            "bpr": b_proj,
            "temp": temperature,
        })
    return in_maps


def kernel(**inputs):
    x = inputs["x"]
    in_maps = prep_host_inputs(
        x, inputs["W_qkv"], inputs["temperature"], inputs["W_proj"], inputs["b_proj"]
    )
    nc = _get_nc()
    res = run_bass_kernel_spmd(nc, in_maps, list(range(B)))
    y = np.stack([np.asarray(res.results[i]["y"]) for i in range(B)], axis=0)
    return y.astype(np.float32)


if __name__ == "__main__":
    # smoke test with random data
    rng = np.random.default_rng(0)
    ins = {
        "x": rng.standard_normal((B, N, C), dtype=np.float32),
        "x_out": rng.standard_normal((B, N, C), dtype=np.float32),
        "W_qkv": (rng.standard_normal((C, 3 * C), dtype=np.float32) / np.sqrt(C)),
        "temperature": np.ones((NH, 1, 1), np.float32),
        "W_proj": (rng.standard_normal((C, C), dtype=np.float32) / np.sqrt(C)),
        "b_proj": rng.standard_normal((C,), dtype=np.float32) * 0.01,
        "H": 64,
        "W": 64,
    }
    out = kernel(**ins)
    print("out", out.shape, out.dtype, float(np.abs(out).max()))


# revision 28
# speedup vs baseline: 1.0298x; 1.0231x over previous
"""CCA (cross-covariance / channel) attention kernel for Trainium2, 8 NeuronCores.

Math (per batch element b, all derived from the reference nn.Module):
    qkv = x @ W_qkv ; per head h: q,k,v in [N, 64] layouts
    channel attention: attn_h = softmax_d( (q_hat^T k_hat) * temp_h ),
    with q_hat = q / ||q||_col (L2 over N), out = attn @ v^T, y = out^T @ W_proj + b.

Key factorization used here (N=4096 >> C=512):
    S = x^T x                      [512,512]   (shared across heads)
    qk_h = Wq_h^T S Wk_h,  |q_c|^2 = diag(Wq_h^T S Wq_h)  (via T = S @ Wqk)
    M_h = attn_h^T Wp_h            [64,512]
    P   = sum_h Wv_h M_h           [512,512]
    y   = x @ P + b                 (big matmul, uses host-pretransposed x^T)

The whole S->T->qk->softmax path is scale-invariant (the cosine
normalization cancels any uniform scale on S), so it runs in fp8
DoubleRow end to end: x is pre-scaled by 1/8 on the host so S/64 falls
out of PSUM in fp8 range with no on-chip rescale.  The y = x @ P matmul
feeds the output directly, so it stays bf16.

Norms use a 64-row ones matmul so n^2 lands broadcast across 64
partitions; r = sqrt(1/n^2) then runs on wide tiles (DVE reciprocal +
ACT sqrt) and rk is just a strided view of the result - no
single-partition Ln/Exp rows.

Data-parallel over B=8 across the 8 cores; no collectives.
"""

import os
import sys
import numpy as np

for _p in ("/opt/trn_rl_repo",):
    if _p not in sys.path and os.path.isdir(_p):
        sys.path.insert(0, _p)

import ml_dtypes  # noqa: E402
from contextlib import ExitStack  # noqa: E402

import functools  # noqa: E402

import concourse.bass as bass  # noqa: E402
import concourse.bacc as bacc  # noqa: E402
import concourse.hw_specs as hw_specs  # noqa: E402


@functools.cache
def _patched_act_tables(arch):
    # Keep Ln/Exp only in natural_log_exp_and_others so the table-load pass
    # resolves both to ONE set (a single ~1.3us ACT_TABLE_LOAD per kernel).
    base = hw_specs.get_activation_tables(arch)
    out = {}
    for name, fns in base.items():
        fns = set(fns)
        if name != "natural_log_exp_and_others":
            fns -= {mybir.ActivationFunctionType.Ln, mybir.ActivationFunctionType.Exp}
        out[name] = fns
    return out


bacc.get_activation_tables = _patched_act_tables
import concourse.tile as tile  # noqa: E402
from concourse import mybir  # noqa: E402
from concourse.bass_utils import run_bass_kernel_spmd  # noqa: E402
from concourse.tile_rust import add_dep_helper  # noqa: E402

B, N, C = 8, 4096, 512
NH, HD = 8, 64
NT = N // 128  # 32 n-tiles
KC = C // 128  # 4 contraction chunks of 128
GP = NH // 2   # 4 head-pairs for the P phase
F32 = mybir.dt.float32
BF16 = mybir.dt.bfloat16
FP8 = mybir.dt.float8e4
AF = mybir.ActivationFunctionType
ALU = mybir.AluOpType
DR = mybir.MatmulPerfMode.DoubleRow
BF16_NP = ml_dtypes.bfloat16
FP8_NP = ml_dtypes.float8_e4m3


def _build_kernel_body(ctx: ExitStack, tc: tile.TileContext, io: dict):
    nc = tc.nc
    x_nat, x_tr, wqk8, wqkb, wv2, wp2, bpr, temp, y = (
        io["x_nat"], io["x_tr"], io["wqk8"], io["wqkb"], io["wv2"],
        io["wp2"], io["bpr"], io["temp"], io["y"],
    )

    persist = ctx.enter_context(tc.tile_pool(name="persist", bufs=1))
    ypool = ctx.enter_context(tc.tile_pool(name="ypool", bufs=6))
    psum = ctx.enter_context(tc.tile_pool(name="psum", bufs=6, space="PSUM"))
    psum_g = ctx.enter_context(tc.tile_pool(name="psum_g", bufs=1, space="PSUM"))

    # ---- PE prewarm (emitted first so the tensor queue ramps the clock
    # while the first x chunk is still in flight) ---------------------------
    scr_sb = persist.tile([128, C], BF16)
    nc.vector.memset(scr_sb, 1.0)
    for i in range(4):
        kp = psum.tile([128, C], F32, tag="work_ps", name=f"prewarm{i}")
        nc.tensor.matmul(kp, scr_sb[:, 0:128], scr_sb, start=True, stop=True)

    # ---- loads -------------------------------------------------------------
    # x (fp8, pre-scaled by 1/8, feeds only S) is host-pre-tiled to
    # [128, NT, C]; streamed in 4 chunks spread across all three DMA queues
    # (2 HWDGE + gpsimd SWDGE) since a single queue runs at ~90 GB/s.
    CHUNK_TILES = [4, 8, 8, 12]
    CHUNK_ENG = [nc.sync, nc.scalar, nc.gpsimd, nc.sync]
    NCHUNK = len(CHUNK_TILES)
    x_chunks = []
    x_dmas = []
    t0 = 0
    for c, ntc in enumerate(CHUNK_TILES):
        xc = persist.tile([128, ntc, C], FP8, tag=f"x_chunk{c}")
        x_dmas.append(CHUNK_ENG[c].dma_start(out=xc, in_=x_nat[:, t0:t0 + ntc, :]))
        x_chunks.append(xc)
        t0 += ntc
    wqk8_sb = persist.tile([128, KC, 2 * C], FP8)
    nc.scalar.dma_start(out=wqk8_sb, in_=wqk8[:])
    ident128 = persist.tile([128, 128], FP8)
    nc.scalar.dma_start(out=ident128, in_=io["ident"][:])
    wqkb_sb = persist.tile([128, KC, 2 * C], BF16)
    nc.gpsimd.dma_start(out=wqkb_sb, in_=wqkb[:])
    wv2_sb = persist.tile([128, GP, C], BF16)  # [(two,d), pair, ci]
    nc.gpsimd.dma_start(out=wv2_sb, in_=wv2[:])
    wp_sb = persist.tile([64, NH, C], BF16)  # [c, (h, e)]
    nc.gpsimd.dma_start(out=wp_sb, in_=wp2[:])
    bias_sb = persist.tile([128, C], F32)
    nc.gpsimd.dma_start(
        out=bias_sb,
        in_=bass.AP(tensor=bpr[:].tensor, offset=bpr[:].offset, ap=[[0, 128], [1, C]]),
    )
    temp_b = persist.tile([64, NH], F32)
    nc.gpsimd.dma_start(
        out=temp_b,
        in_=bass.AP(tensor=temp[:].tensor, offset=temp[:].offset,
                    ap=[[0, 64], [1, NH]]),
    )
    ones_bc = persist.tile([128, HD], BF16)  # 64-wide ones: norms broadcast
    nc.vector.memset(ones_bc, 1.0)
    # xT (bf16, feeds only the y phase) streams behind the x chunks across
    # all three queues.  Lands well before the y phase needs it.
    xt_sb = persist.tile([128, KC, N], BF16)
    xt_view = x_tr[:].rearrange("(k p) n -> p k n", p=128)
    XT_ENG = [nc.sync, nc.scalar, nc.gpsimd]
    for g in range(6):
        xd = XT_ENG[g % 3].dma_start(
            out=xt_sb[:, :, g * 683:min(N, (g + 1) * 683)],
            in_=xt_view[:, :, g * 683:min(N, (g + 1) * 683)],
        )
        add_dep_helper(xd.ins, x_dmas[-1].ins,
                       reason="xT load deferred behind S inputs")

    # ACT table warmup (Exp for softmax; emitted early so the table load
    # happens during the DMA ramp).
    warm_sb = persist.tile([1, 2], F32)
    nc.vector.memset(warm_sb, 1.0)
    nc.scalar.activation(warm_sb[:, 1:2], warm_sb[:, 1:2], AF.Exp)
    nc.scalar.activation(warm_sb[:, 0:1], warm_sb[:, 0:1], AF.Ln)

    # small dependency-paced PE keepalive for the softmax/norms lulls: keep()
    # waits on the chain tensor, dense() adds real PE density behind it.
    _keep_n = [0]

    def keep(dep):
        kp = psum.tile([1, 2], F32, tag="work_ps", name=f"keep{_keep_n[0]}")
        _keep_n[0] += 1
        nc.tensor.matmul(kp[:, 0:1], dep, dep, start=True, stop=True)

    def dense(n):
        for _ in range(n):
            kp = psum.tile([128, C], F32, tag="work_ps", name=f"dense{_keep_n[0]}")
            _keep_n[0] += 1
            nc.tensor.matmul(
                kp, wqk8_sb[:, 0, 0:128], wqk8_sb[:, 0, 0:C], start=True, stop=True
            )

    # ---- S = (x/8)^T (x/8) = S_true/64  [C, C], fp8 DoubleRow -------------
    # S is symmetric: compute only the upper-triangle block-rows (rhs width
    # shrinks 512/384/256/128) and mirror the 6 lower blocks via PE
    # transposes.  Chunk-outer loop so accumulation starts when the first x
    # chunk arrives; the last chunk goes kc-sequential so each s8 copy
    # overlaps the next bank's remaining matmuls.
    s8_sb = persist.tile([128, KC, C], FP8)
    s_ps = [
        psum.tile([128, C - 128 * kc], F32, tag="work_ps", name=f"s_ps{kc}")
        for kc in range(KC)
    ]
    for c in range(NCHUNK - 1):
        for kc in range(KC):
            for tp in range(CHUNK_TILES[c] // 2):
                nc.tensor.matmul(
                    s_ps[kc],
                    x_chunks[c][:, 2 * tp:2 * tp + 2, kc * 128:(kc + 1) * 128],
                    x_chunks[c][:, 2 * tp:2 * tp + 2, kc * 128:],
                    perf_mode=DR,
                    start=(c == 0 and tp == 0),
                    stop=False,
                )
    for kc in range(KC):
        cl = NCHUNK - 1
        for tp in range(CHUNK_TILES[cl] // 2):
            nc.tensor.matmul(
                s_ps[kc],
                x_chunks[cl][:, 2 * tp:2 * tp + 2, kc * 128:(kc + 1) * 128],
                x_chunks[cl][:, 2 * tp:2 * tp + 2, kc * 128:],
                perf_mode=DR,
                start=False,
                stop=(tp == CHUNK_TILES[cl] // 2 - 1),
            )
        # S -> fp8 (no rescale needed; host pre-scaled x); overlaps the next
        # bank's chunk-3 matmuls.
        if kc % 2 == 0:
            nc.scalar.copy(s8_sb[:, kc, kc * 128:], s_ps[kc])
        else:
            nc.vector.tensor_copy(s8_sb[:, kc, kc * 128:], s_ps[kc])
    # mirror the lower-triangle blocks: s8[j, i] = s8[i, j]^T
    nmir = 0
    for i in range(KC):
        for j in range(i + 1, KC):
            # fp8 transpose writes with an output element step of 2
            mir_ps = psum.tile([128, 128, 2], FP8, tag="work_ps", name=f"mir{i}{j}")
            mir_v = mir_ps[:, :, 0]
            nc.tensor.transpose(
                mir_v, s8_sb[:, i, j * 128:(j + 1) * 128], ident128
            )
            if nmir % 2 == 0:
                nc.scalar.copy(s8_sb[:, j, i * 128:(i + 1) * 128], mir_v)
            else:
                nc.vector.tensor_copy(s8_sb[:, j, i * 128:(i + 1) * 128], mir_v)
            nmir += 1

    # ---- T = S8 @ Wqk8 [C, 2C] in fp8 DoubleRow ---------------------------
    # Only the k-half of T feeds the qk matmuls (fp8 copies); pn = Wqk*T
    # (both halves, read straight from PSUM) feeds the norms reduction and
    # runs on the otherwise-idle gpsimd engine.
    t8_sb = persist.tile([128, KC, C], FP8)  # k-half of T only
    pn_sb = persist.tile([128, KC, 2 * C], BF16)
    for ti in range(KC):
        for half in range(2):
            t_ps = psum.tile([128, C], F32, tag="work_ps")
            for jp in range(2):
                nc.tensor.matmul(
                    t_ps,
                    s8_sb[:, 2 * jp:2 * jp + 2, ti * 128:(ti + 1) * 128],
                    wqk8_sb[:, 2 * jp:2 * jp + 2, half * C:(half + 1) * C],
                    perf_mode=DR,
                    start=(jp == 0),
                    stop=(jp == 1),
                )
            # pn on DVE (gpsimd can't read PSUM), t8 k-copies on ACT
            nc.vector.tensor_mul(
                pn_sb[:, ti, half * C:(half + 1) * C],
                wqkb_sb[:, ti, half * C:(half + 1) * C],
                t_ps,
            )
            # one strided copy pulls this half's 4 per-head k-col blocks
            # ([hh*128+64 : hh*128+128]) into t8 packed as (h, d)
            ksrc = t_ps.rearrange("p (hh s) -> p hh s", s=128)[:, :, HD:]
            kdst = t8_sb[:, ti, half * 4 * HD:(half + 1) * 4 * HD].rearrange(
                "p (hh d) -> p hh d", d=HD
            )
            nc.scalar.copy(kdst, ksrc)

    # ---- qk_h = Wq8_h^T T8_k(h)  [64, 64] per head, fp8 DoubleRow ---------
    # (head-outer, pair-inner: PSUM accumulation groups are bank-scoped)
    qk_ps = psum_g.tile([64, NH, HD], F32)
    for h in range(NH):
        for jp in range(2):
            nc.tensor.matmul(
                qk_ps[:, h, :],
                wqk8_sb[:, 2 * jp:2 * jp + 2, h * 128:h * 128 + HD],
                t8_sb[:, 2 * jp:2 * jp + 2, h * HD:(h + 1) * HD],
                perf_mode=DR,
                start=(jp == 0),
                stop=(jp == 1),
            )

    # ---- norms: n2 broadcast via 64-row ones matmuls, r = sqrt(1/n2) ------
    # nrm_ps[half][c, w] = n2[w] for every c<64; rk is then just a strided
    # view of r_bc and rq comes from 8 tiny PE transposes of row 0.
    nrm_ps = [
        psum.tile([64, C], F32, tag="work_ps", name=f"nrm_ps{half}")
        for half in range(2)
    ]
    for half in range(2):
        for ti in range(KC):
            nc.tensor.matmul(
                nrm_ps[half],
                ones_bc,
                pn_sb[:, ti, half * C:(half + 1) * C],
                start=(ti == 0),
                stop=(ti == KC - 1),
            )
    lnr_bc = persist.tile([64, 2 * C], F32)
    for half in range(2):
        nc.scalar.activation(
            lnr_bc[:, half * C:(half + 1) * C], nrm_ps[half], AF.Ln
        )
    r_bc = persist.tile([64, 2 * C], BF16)  # [c, (h, {q64|k64})] broadcast
    nc.scalar.activation(r_bc, lnr_bc, AF.Exp, scale=-0.5)
    keep(r_bc[0:1, 0:1])
    dense(2)

    # rq: r_bc's q-slices transposed to the partition dim, * temperature
    ident1 = persist.tile([1, 1], BF16)
    nc.vector.memset(ident1, 1.0)
    tr_ps = psum.tile([64, 2 * NH], BF16, tag="work_ps")
    for h in range(NH):
        nc.tensor.transpose(
            tr_ps[:, 2 * h:2 * h + 1], r_bc[0:1, h * 128:h * 128 + HD], ident1
        )
    rq_sb = persist.tile([64, NH], F32)
    tr_view = tr_ps.rearrange("p (h two) -> p h two", two=2)[:, :, 0]
    nc.vector.tensor_mul(rq_sb, tr_view, temp_b)  # fold temperature into rq
    rk_view = r_bc.rearrange("p (h s) -> p h s", s=128)[:, :, HD:]  # [64,NH,HD]

    # ---- softmax (all heads fused) -> M -> P (head-pair packed) -----------
    # |logits| <= max(temperature) so exp() is safe without max-subtraction.
    #
    # Engines can't shift partitions, so the pair packing happens inside the
    # M matmuls: attn is written into a zero-padded stationary layout
    # attn_pad[:, j, two, two*64:(two+1)*64] and the two matmuls of pair j
    # accumulate M_even into PSUM partitions 0:64 and M_odd into 64:128 of
    # one [128, C] tile.  P then contracts 128 rows (2 heads) per matmul.
    lg = persist.tile([64, NH, HD], F32)
    ex = persist.tile([64, NH, HD], F32)
    ssum = persist.tile([64, NH], F32)
    attn_pad = persist.tile([64, GP, 2, 2 * HD], BF16)
    nc.vector.memset(attn_pad, 0.0)
    m2_sb = persist.tile([128, GP, C], BF16)  # [(two,d), pair, e]
    p_ps = [
        psum.tile([128, C], F32, tag="work_ps", name=f"p_ps{t}") for t in range(KC)
    ]

    nc.vector.tensor_mul(lg, qk_ps, rk_view)
    nc.vector.tensor_mul(
        lg, lg, rq_sb[:, :, None].broadcast_to([64, NH, HD])
    )
    keep(lg[0:1, NH - 1, 0:1])
    dense(1)
    nc.scalar.activation(ex, lg, AF.Exp)
    keep(ex[0:1, NH - 1, 0:1])
    dense(1)
    nc.vector.tensor_reduce(
        ssum[:, :, None], ex, axis=mybir.AxisListType.X, op=ALU.add
    )
    nc.vector.reciprocal(ssum, ssum)
    # evens -> attn_pad[:, :, 0, 0:64], odds -> attn_pad[:, :, 1, 64:128]
    ex_v = ex.rearrange("p (j two) d -> p j two d", two=2)
    ss_v = ssum.rearrange("p (j two) -> p j two", two=2)
    nc.vector.tensor_mul(
        attn_pad[:, :, 0, 0:HD],
        ex_v[:, :, 0, :],
        ss_v[:, :, 0, None].broadcast_to([64, GP, HD]),
    )
    nc.vector.tensor_mul(
        attn_pad[:, :, 1, HD:2 * HD],
        ex_v[:, :, 1, :],
        ss_v[:, :, 1, None].broadcast_to([64, GP, HD]),
    )

    def emit_p(j):  # accumulate head-pair j into all four P row-blocks
        for t in range(KC):
            nc.tensor.matmul(
                p_ps[t],
                wv2_sb[:, j, t * 128:(t + 1) * 128],
                m2_sb[:, j, :],
                start=(j == 0),
                stop=(j == GP - 1),
            )

    # M/P software pipeline: P(pair j-1) runs while m2[j] is being filled
    for j in range(GP):
        m2_ps = psum.tile([128, C], F32, tag="work_ps")
        for two in range(2):
            nc.tensor.matmul(
                m2_ps,
                attn_pad[:, j, two, :],
                wp_sb[:, 2 * j + two, :],
                start=(two == 0),
                stop=(two == 1),
            )
        if j % 2 == 0:
            nc.scalar.copy(m2_sb[:, j, :], m2_ps)
        else:
            nc.vector.tensor_copy(m2_sb[:, j, :], m2_ps)
        if j > 0:
            emit_p(j - 1)
    emit_p(GP - 1)

    p_sb = persist.tile([128, KC, C], BF16)
    for t in range(KC):
        if t % 2 == 0:
            nc.scalar.copy(p_sb[:, t, :], p_ps[t])
        else:
            nc.vector.tensor_copy(p_sb[:, t, :], p_ps[t])

    # ---- y = x @ P + b ----------------------------------------------------
    # paired-tile stores: one DMA per two n-tiles, round-robin on all queues.
    y_paired = y[:].rearrange("(t p) c -> p t c", p=128)
    ST_ENG = [nc.sync, nc.scalar, nc.gpsimd]
    ybuf = None
    for t in range(NT):
        y_ps = psum.tile([128, C], F32, tag="work_ps")
        for kc in range(KC):
            nc.tensor.matmul(
                y_ps,
                xt_sb[:, kc, t * 128:(t + 1) * 128],
                p_sb[:, kc, :],
                start=(kc == 0),
                stop=(kc == KC - 1),
            )
        if t >= NT - 2:
            # last two tiles: split bias-add + store into column halves on
            # different queues so the final transfer is quarter-size and the
            # first half is already in flight while the second is added
            ybuf = ypool.tile([128, 1, C], BF16)
            HC = C // 2
            for hcol in range(2):
                nc.vector.tensor_add(
                    ybuf[:, 0, hcol * HC:(hcol + 1) * HC],
                    y_ps[:, hcol * HC:(hcol + 1) * HC],
                    bias_sb[:, hcol * HC:(hcol + 1) * HC],
                )
                ST_ENG[(t + hcol) % 3].dma_start(
                    out=y_paired[:, t:t + 1, hcol * HC:(hcol + 1) * HC],
                    in_=ybuf[:, 0:1, hcol * HC:(hcol + 1) * HC],
                )
        elif t >= NT - 4:
            # single-tile stores near the end so the drain finishes sooner
            ybuf = ypool.tile([128, 1, C], BF16)
            nc.vector.tensor_add(ybuf[:, 0, :], y_ps, bias_sb)
            ST_ENG[t % 3].dma_start(out=y_paired[:, t:t + 1, :], in_=ybuf)
        else:
            if t % 2 == 0:
                ybuf = ypool.tile([128, 2, C], BF16)
            nc.vector.tensor_add(ybuf[:, t % 2, :], y_ps, bias_sb)
            if t % 2 == 1:
                eng = ST_ENG[(t // 2) % 3]
                eng.dma_start(out=y_paired[:, t - 1:t + 1, :], in_=ybuf)


def build_nc():
    nc = bacc.Bacc("TRN2", target_bir_lowering=False, debug=False, num_devices=B)
    io = {}
    io["x_nat"] = nc.dram_tensor("x_nat", [128, NT, C], FP8, kind="ExternalInput")
    io["x_tr"] = nc.dram_tensor("x_tr", [C, N], BF16, kind="ExternalInput")
    io["wqk8"] = nc.dram_tensor("wqk8", [128, KC, 2 * C], FP8, kind="ExternalInput")
    io["ident"] = nc.dram_tensor("ident", [128, 128], FP8, kind="ExternalInput")
    io["wqkb"] = nc.dram_tensor("wqkb", [128, KC, 2 * C], BF16, kind="ExternalInput")
    io["wv2"] = nc.dram_tensor("wv2", [128, GP, C], BF16, kind="ExternalInput")
    io["wp2"] = nc.dram_tensor("wp2", [64, NH, C], BF16, kind="ExternalInput")
    io["bpr"] = nc.dram_tensor("bpr", [C], F32, kind="ExternalInput")
    io["temp"] = nc.dram_tensor("temp", [NH], F32, kind="ExternalInput")
    io["y"] = nc.dram_tensor("y", [N, C], BF16, kind="ExternalOutput")
    with tile.TileContext(nc) as tc:
        with ExitStack() as ctx:
            _build_kernel_body(ctx, tc, io)
    nc.compile()
    return nc


_NC_CACHE = None


def _get_nc():
    global _NC_CACHE
    if _NC_CACHE is None:
        _NC_CACHE = build_nc()
    return _NC_CACHE


def prep_host_inputs(x, W_qkv, temperature, W_proj, b_proj):
    """Host-side preprocessing shared by all cores. Returns per-core in_maps."""
    x = np.asarray(x, dtype=np.float32)
    W_qkv = np.asarray(W_qkv, dtype=np.float32)
    temperature = np.asarray(temperature, dtype=np.float32).reshape(NH)
    W_proj = np.asarray(W_proj, dtype=np.float32)
    b_proj = np.asarray(b_proj, dtype=np.float32)

    Wq = W_qkv[:, 0:C].reshape(C, NH, HD)
    Wk = W_qkv[:, C:2 * C].reshape(C, NH, HD)
    wqk_perm = np.concatenate([Wq, Wk], axis=2).reshape(C, 2 * C)  # [(ci),(h)(qk c)]
    wqk_tiled = np.ascontiguousarray(
        wqk_perm.reshape(KC, 128, 2 * C).transpose(1, 0, 2)
    )  # [p, kc, 2C]
    Wv = W_qkv[:, 2 * C:3 * C]  # [ci, (h d)]
    # [(two,d), pair, ci] so head-pairs stack on the partition dim for P
    wv2 = np.ascontiguousarray(
        Wv.T.reshape(GP, 2, HD, C).transpose(1, 2, 0, 3).reshape(128, GP, C)
    )
    wp2 = np.ascontiguousarray(
        W_proj.reshape(NH, HD, C).transpose(1, 0, 2)
    )  # [c, h, e]

    wqk8 = wqk_tiled.astype(FP8_NP)
    wqkb = wqk_tiled.astype(BF16_NP)
    ident = np.eye(128, dtype=np.float32).astype(FP8_NP)
    wv2_bf = wv2.astype(BF16_NP)
    wp2_bf = wp2.astype(BF16_NP)

    in_maps = []
    for b in range(B):
        xb = x[b]
        in_maps.append({
            "x_nat": np.ascontiguousarray(
                (xb * 0.125).reshape(NT, 128, C).transpose(1, 0, 2)
            ).astype(FP8_NP),
            "x_tr": np.ascontiguousarray(xb.T).astype(BF16_NP),
            "wqk8": wqk8,
            "ident": ident,
            "wqkb": wqkb,
            "wv2": wv2_bf,
            "wp2": wp2_bf,
            "bpr": b_proj,
            "temp": temperature,
        })
    return in_maps


def kernel(**inputs):
    x = inputs["x"]
    in_maps = prep_host_inputs(
        x, inputs["W_qkv"], inputs["temperature"], inputs["W_proj"], inputs["b_proj"]
    )
    nc = _get_nc()
    res = run_bass_kernel_spmd(nc, in_maps, list(range(B)))
    y = np.stack([np.asarray(res.results[i]["y"]) for i in range(B)], axis=0)
    return y.astype(np.float32)


if __name__ == "__main__":
    # smoke test with random data
    rng = np.random.default_rng(0)
    ins = {
        "x": rng.standard_normal((B, N, C), dtype=np.float32),
        "x_out": rng.standard_normal((B, N, C), dtype=np.float32),
        "W_qkv": (rng.standard_normal((C, 3 * C), dtype=np.float32) / np.sqrt(C)),
        "temperature": np.ones((NH, 1, 1), np.float32),
        "W_proj": (rng.standard_normal((C, C), dtype=np.float32) / np.sqrt(C)),
        "b_proj": rng.standard_normal((C,), dtype=np.float32) * 0.01,
        "H": 64,
        "W": 64,
    }
    out = kernel(**ins)
    print("out", out.shape, out.dtype, float(np.abs(out).max()))


# revision 29
# speedup vs baseline: 1.0332x; 1.0033x over previous
"""CCA (cross-covariance / channel) attention kernel for Trainium2, 8 NeuronCores.

Math (per batch element b, all derived from the reference nn.Module):
    qkv = x @ W_qkv ; per head h: q,k,v in [N, 64] layouts
    channel attention: attn_h = softmax_d( (q_hat^T k_hat) * temp_h ),
    with q_hat = q / ||q||_col (L2 over N), out = attn @ v^T, y = out^T @ W_proj + b.

Key factorization used here (N=4096 >> C=512):
    S = x^T x                      [512,512]   (shared across heads)
    qk_h = Wq_h^T S Wk_h,  |q_c|^2 = diag(Wq_h^T S Wq_h)  (via T = S @ Wqk)
    M_h = attn_h^T Wp_h            [64,512]
    P   = sum_h Wv_h M_h           [512,512]
    y   = x @ P + b                 (big matmul, uses host-pretransposed x^T)

The whole S->T->qk->softmax path is scale-invariant (the cosine
normalization cancels any uniform scale on S), so it runs in fp8
DoubleRow end to end: x is pre-scaled by 1/8 on the host so S/64 falls
out of PSUM in fp8 range with no on-chip rescale.  The y = x @ P matmul
feeds the output directly, so it stays bf16.

Norms use a 64-row ones matmul so n^2 lands broadcast across 64
partitions; r = sqrt(1/n^2) then runs on wide tiles (DVE reciprocal +
ACT sqrt) and rk is just a strided view of the result - no
single-partition Ln/Exp rows.

Data-parallel over B=8 across the 8 cores; no collectives.
"""

import os
import sys
import numpy as np

for _p in ("/opt/trn_rl_repo",):
    if _p not in sys.path and os.path.isdir(_p):
        sys.path.insert(0, _p)

import ml_dtypes  # noqa: E402
from contextlib import ExitStack  # noqa: E402

import functools  # noqa: E402

import concourse.bass as bass  # noqa: E402
import concourse.bacc as bacc  # noqa: E402
import concourse.hw_specs as hw_specs  # noqa: E402


@functools.cache
def _patched_act_tables(arch):
    # Keep Ln/Exp only in natural_log_exp_and_others so the table-load pass
    # resolves both to ONE set (a single ~1.3us ACT_TABLE_LOAD per kernel).
    base = hw_specs.get_activation_tables(arch)
    out = {}
    for name, fns in base.items():
        fns = set(fns)
        if name != "natural_log_exp_and_others":
            fns -= {mybir.ActivationFunctionType.Ln, mybir.ActivationFunctionType.Exp}
        out[name] = fns
    return out


bacc.get_activation_tables = _patched_act_tables
import concourse.tile as tile  # noqa: E402
from concourse import mybir  # noqa: E402
from concourse.bass_utils import run_bass_kernel_spmd  # noqa: E402
from concourse.tile_rust import add_dep_helper  # noqa: E402

B, N, C = 8, 4096, 512
NH, HD = 8, 64
NT = N // 128  # 32 n-tiles
KC = C // 128  # 4 contraction chunks of 128
GP = NH // 2   # 4 head-pairs for the P phase
F32 = mybir.dt.float32
BF16 = mybir.dt.bfloat16
FP8 = mybir.dt.float8e4
AF = mybir.ActivationFunctionType
ALU = mybir.AluOpType
DR = mybir.MatmulPerfMode.DoubleRow
BF16_NP = ml_dtypes.bfloat16
FP8_NP = ml_dtypes.float8_e4m3


def _build_kernel_body(ctx: ExitStack, tc: tile.TileContext, io: dict):
    nc = tc.nc
    x_nat, x_tr, wqk8, wqkb, wv2, wp2, bpr, temp, y = (
        io["x_nat"], io["x_tr"], io["wqk8"], io["wqkb"], io["wv2"],
        io["wp2"], io["bpr"], io["temp"], io["y"],
    )

    persist = ctx.enter_context(tc.tile_pool(name="persist", bufs=1))
    ypool = ctx.enter_context(tc.tile_pool(name="ypool", bufs=6))
    psum = ctx.enter_context(tc.tile_pool(name="psum", bufs=6, space="PSUM"))
    psum_g = ctx.enter_context(tc.tile_pool(name="psum_g", bufs=1, space="PSUM"))

    # ---- PE prewarm (emitted first so the tensor queue ramps the clock
    # while the first x chunk is still in flight) ---------------------------
    scr_sb = persist.tile([128, C], BF16)
    nc.vector.memset(scr_sb, 1.0)
    for i in range(4):
        kp = psum.tile([128, C], F32, tag="work_ps", name=f"prewarm{i}")
        nc.tensor.matmul(kp, scr_sb[:, 0:128], scr_sb, start=True, stop=True)

    # ---- loads -------------------------------------------------------------
    # x (fp8, pre-scaled by 1/8, feeds only S) is host-pre-tiled to
    # [128, NT, C]; streamed in 4 chunks spread across all three DMA queues
    # (2 HWDGE + gpsimd SWDGE) since a single queue runs at ~90 GB/s.
    CHUNK_TILES = [4, 8, 8, 12]
    CHUNK_ENG = [nc.sync, nc.scalar, nc.sync, nc.gpsimd]
    NCHUNK = len(CHUNK_TILES)
    x_chunks = []
    x_dmas = []
    t0 = 0
    for c, ntc in enumerate(CHUNK_TILES):
        xc = persist.tile([128, ntc, C], FP8, tag=f"x_chunk{c}")
        x_dmas.append(CHUNK_ENG[c].dma_start(out=xc, in_=x_nat[:, t0:t0 + ntc, :]))
        x_chunks.append(xc)
        t0 += ntc
    wqk8_sb = persist.tile([128, KC, 2 * C], FP8)
    nc.scalar.dma_start(out=wqk8_sb, in_=wqk8[:])
    ident128 = persist.tile([128, 128], FP8)
    nc.scalar.dma_start(out=ident128, in_=io["ident"][:])
    wqkb_sb = persist.tile([128, KC, 2 * C], BF16)
    nc.gpsimd.dma_start(out=wqkb_sb, in_=wqkb[:])
    wv2_sb = persist.tile([128, GP, C], BF16)  # [(two,d), pair, ci]
    nc.gpsimd.dma_start(out=wv2_sb, in_=wv2[:])
    wp_sb = persist.tile([64, NH, C], BF16)  # [c, (h, e)]
    nc.gpsimd.dma_start(out=wp_sb, in_=wp2[:])
    bias_sb = persist.tile([128, C], F32)
    nc.gpsimd.dma_start(
        out=bias_sb,
        in_=bass.AP(tensor=bpr[:].tensor, offset=bpr[:].offset, ap=[[0, 128], [1, C]]),
    )
    temp_b = persist.tile([64, NH], F32)
    nc.gpsimd.dma_start(
        out=temp_b,
        in_=bass.AP(tensor=temp[:].tensor, offset=temp[:].offset,
                    ap=[[0, 64], [1, NH]]),
    )
    ones_bc = persist.tile([128, HD], BF16)  # 64-wide ones: norms broadcast
    nc.vector.memset(ones_bc, 1.0)
    # xT (bf16, feeds only the y phase) streams behind the x chunks across
    # all three queues.  Lands well before the y phase needs it.
    xt_sb = persist.tile([128, KC, N], BF16)
    xt_view = x_tr[:].rearrange("(k p) n -> p k n", p=128)
    XT_ENG = [nc.sync, nc.scalar, nc.gpsimd]
    for g in range(6):
        xd = XT_ENG[g % 3].dma_start(
            out=xt_sb[:, :, g * 683:min(N, (g + 1) * 683)],
            in_=xt_view[:, :, g * 683:min(N, (g + 1) * 683)],
        )
        add_dep_helper(xd.ins, x_dmas[-1].ins,
                       reason="xT load deferred behind S inputs")

    # ACT table warmup (Exp for softmax; emitted early so the table load
    # happens during the DMA ramp).
    warm_sb = persist.tile([1, 2], F32)
    nc.vector.memset(warm_sb, 1.0)
    nc.scalar.activation(warm_sb[:, 1:2], warm_sb[:, 1:2], AF.Exp)
    nc.scalar.activation(warm_sb[:, 0:1], warm_sb[:, 0:1], AF.Ln)

    # small dependency-paced PE keepalive for the softmax/norms lulls: keep()
    # waits on the chain tensor, dense() adds real PE density behind it.
    _keep_n = [0]

    def keep(dep):
        kp = psum.tile([1, 2], F32, tag="work_ps", name=f"keep{_keep_n[0]}")
        _keep_n[0] += 1
        nc.tensor.matmul(kp[:, 0:1], dep, dep, start=True, stop=True)

    def dense(n):
        for _ in range(n):
            kp = psum.tile([128, C], F32, tag="work_ps", name=f"dense{_keep_n[0]}")
            _keep_n[0] += 1
            nc.tensor.matmul(
                kp, wqk8_sb[:, 0, 0:128], wqk8_sb[:, 0, 0:C], start=True, stop=True
            )

    # ---- S = (x/8)^T (x/8) = S_true/64  [C, C], fp8 DoubleRow -------------
    # S is symmetric: compute only the upper-triangle block-rows (rhs width
    # shrinks 512/384/256/128) and mirror the 6 lower blocks via PE
    # transposes.  Chunk-outer loop so accumulation starts when the first x
    # chunk arrives; the last chunk goes kc-sequential so each s8 copy
    # overlaps the next bank's remaining matmuls.
    s8_sb = persist.tile([128, KC, C], FP8)
    s_ps = [
        psum.tile([128, C - 128 * kc], F32, tag="work_ps", name=f"s_ps{kc}")
        for kc in range(KC)
    ]
    for c in range(NCHUNK - 1):
        for kc in range(KC):
            for tp in range(CHUNK_TILES[c] // 2):
                nc.tensor.matmul(
                    s_ps[kc],
                    x_chunks[c][:, 2 * tp:2 * tp + 2, kc * 128:(kc + 1) * 128],
                    x_chunks[c][:, 2 * tp:2 * tp + 2, kc * 128:],
                    perf_mode=DR,
                    start=(c == 0 and tp == 0),
                    stop=False,
                )
    for kc in range(KC):
        cl = NCHUNK - 1
        for tp in range(CHUNK_TILES[cl] // 2):
            nc.tensor.matmul(
                s_ps[kc],
                x_chunks[cl][:, 2 * tp:2 * tp + 2, kc * 128:(kc + 1) * 128],
                x_chunks[cl][:, 2 * tp:2 * tp + 2, kc * 128:],
                perf_mode=DR,
                start=False,
                stop=(tp == CHUNK_TILES[cl] // 2 - 1),
            )
        # S -> fp8 (no rescale needed; host pre-scaled x); overlaps the next
        # bank's chunk-3 matmuls.
        if kc % 2 == 0:
            nc.scalar.copy(s8_sb[:, kc, kc * 128:], s_ps[kc])
        else:
            nc.vector.tensor_copy(s8_sb[:, kc, kc * 128:], s_ps[kc])
    # mirror the lower-triangle blocks: s8[j, i] = s8[i, j]^T
    nmir = 0
    for i in range(KC):
        for j in range(i + 1, KC):
            # fp8 transpose writes with an output element step of 2
            mir_ps = psum.tile([128, 128, 2], FP8, tag="work_ps", name=f"mir{i}{j}")
            mir_v = mir_ps[:, :, 0]
            nc.tensor.transpose(
                mir_v, s8_sb[:, i, j * 128:(j + 1) * 128], ident128
            )
            if nmir % 2 == 0:
                nc.scalar.copy(s8_sb[:, j, i * 128:(i + 1) * 128], mir_v)
            else:
                nc.vector.tensor_copy(s8_sb[:, j, i * 128:(i + 1) * 128], mir_v)
            nmir += 1

    # ---- T = S8 @ Wqk8 [C, 2C] in fp8 DoubleRow ---------------------------
    # Only the k-half of T feeds the qk matmuls (fp8 copies); pn = Wqk*T
    # (both halves, read straight from PSUM) feeds the norms reduction and
    # runs on the otherwise-idle gpsimd engine.
    t8_sb = persist.tile([128, KC, C], FP8)  # k-half of T only
    pn_sb = persist.tile([128, KC, 2 * C], BF16)
    for ti in range(KC):
        for half in range(2):
            t_ps = psum.tile([128, C], F32, tag="work_ps")
            for jp in range(2):
                nc.tensor.matmul(
                    t_ps,
                    s8_sb[:, 2 * jp:2 * jp + 2, ti * 128:(ti + 1) * 128],
                    wqk8_sb[:, 2 * jp:2 * jp + 2, half * C:(half + 1) * C],
                    perf_mode=DR,
                    start=(jp == 0),
                    stop=(jp == 1),
                )
            # pn on DVE (gpsimd can't read PSUM), t8 k-copies on ACT
            nc.vector.tensor_mul(
                pn_sb[:, ti, half * C:(half + 1) * C],
                wqkb_sb[:, ti, half * C:(half + 1) * C],
                t_ps,
            )
            # one strided copy pulls this half's 4 per-head k-col blocks
            # ([hh*128+64 : hh*128+128]) into t8 packed as (h, d)
            ksrc = t_ps.rearrange("p (hh s) -> p hh s", s=128)[:, :, HD:]
            kdst = t8_sb[:, ti, half * 4 * HD:(half + 1) * 4 * HD].rearrange(
                "p (hh d) -> p hh d", d=HD
            )
            nc.scalar.copy(kdst, ksrc)

    # ---- qk_h = Wq8_h^T T8_k(h)  [64, 64] per head, fp8 DoubleRow ---------
    # (head-outer, pair-inner: PSUM accumulation groups are bank-scoped)
    qk_ps = psum_g.tile([64, NH, HD], F32)
    for h in range(NH):
        for jp in range(2):
            nc.tensor.matmul(
                qk_ps[:, h, :],
                wqk8_sb[:, 2 * jp:2 * jp + 2, h * 128:h * 128 + HD],
                t8_sb[:, 2 * jp:2 * jp + 2, h * HD:(h + 1) * HD],
                perf_mode=DR,
                start=(jp == 0),
                stop=(jp == 1),
            )

    # ---- norms: n2 broadcast via 64-row ones matmuls, r = sqrt(1/n2) ------
    # nrm_ps[half][c, w] = n2[w] for every c<64; rk is then just a strided
    # view of r_bc and rq comes from 8 tiny PE transposes of row 0.
    nrm_ps = [
        psum.tile([64, C], F32, tag="work_ps", name=f"nrm_ps{half}")
        for half in range(2)
    ]
    for half in range(2):
        for ti in range(KC):
            nc.tensor.matmul(
                nrm_ps[half],
                ones_bc,
                pn_sb[:, ti, half * C:(half + 1) * C],
                start=(ti == 0),
                stop=(ti == KC - 1),
            )
    lnr_bc = persist.tile([64, 2 * C], F32)
    for half in range(2):
        nc.scalar.activation(
            lnr_bc[:, half * C:(half + 1) * C], nrm_ps[half], AF.Ln
        )
    r_bc = persist.tile([64, 2 * C], BF16)  # [c, (h, {q64|k64})] broadcast
    nc.scalar.activation(r_bc, lnr_bc, AF.Exp, scale=-0.5)
    keep(r_bc[0:1, 0:1])
    dense(2)

    # rq: r_bc's q-slices transposed to the partition dim, * temperature
    ident1 = persist.tile([1, 1], BF16)
    nc.vector.memset(ident1, 1.0)
    tr_ps = psum.tile([64, 2 * NH], BF16, tag="work_ps")
    for h in range(NH):
        nc.tensor.transpose(
            tr_ps[:, 2 * h:2 * h + 1], r_bc[0:1, h * 128:h * 128 + HD], ident1
        )
    rq_sb = persist.tile([64, NH], F32)
    tr_view = tr_ps.rearrange("p (h two) -> p h two", two=2)[:, :, 0]
    nc.vector.tensor_mul(rq_sb, tr_view, temp_b)  # fold temperature into rq
    rk_view = r_bc.rearrange("p (h s) -> p h s", s=128)[:, :, HD:]  # [64,NH,HD]

    # ---- softmax (all heads fused) -> M -> P (head-pair packed) -----------
    # |logits| <= max(temperature) so exp() is safe without max-subtraction.
    #
    # Engines can't shift partitions, so the pair packing happens inside the
    # M matmuls: attn is written into a zero-padded stationary layout
    # attn_pad[:, j, two, two*64:(two+1)*64] and the two matmuls of pair j
    # accumulate M_even into PSUM partitions 0:64 and M_odd into 64:128 of
    # one [128, C] tile.  P then contracts 128 rows (2 heads) per matmul.
    lg = persist.tile([64, NH, HD], F32)
    ex = persist.tile([64, NH, HD], F32)
    ssum = persist.tile([64, NH], F32)
    attn_pad = persist.tile([64, GP, 2, 2 * HD], BF16)
    nc.vector.memset(attn_pad, 0.0)
    m2_sb = persist.tile([128, GP, C], BF16)  # [(two,d), pair, e]
    p_ps = [
        psum.tile([128, C], F32, tag="work_ps", name=f"p_ps{t}") for t in range(KC)
    ]

    nc.vector.tensor_mul(lg, qk_ps, rk_view)
    nc.vector.tensor_mul(
        lg, lg, rq_sb[:, :, None].broadcast_to([64, NH, HD])
    )
    keep(lg[0:1, NH - 1, 0:1])
    dense(1)
    nc.scalar.activation(ex, lg, AF.Exp)
    keep(ex[0:1, NH - 1, 0:1])
    dense(1)
    nc.vector.tensor_reduce(
        ssum[:, :, None], ex, axis=mybir.AxisListType.X, op=ALU.add
    )
    nc.vector.reciprocal(ssum, ssum)
    # evens -> attn_pad[:, :, 0, 0:64], odds -> attn_pad[:, :, 1, 64:128]
    ex_v = ex.rearrange("p (j two) d -> p j two d", two=2)
    ss_v = ssum.rearrange("p (j two) -> p j two", two=2)
    nc.vector.tensor_mul(
        attn_pad[:, :, 0, 0:HD],
        ex_v[:, :, 0, :],
        ss_v[:, :, 0, None].broadcast_to([64, GP, HD]),
    )
    nc.vector.tensor_mul(
        attn_pad[:, :, 1, HD:2 * HD],
        ex_v[:, :, 1, :],
        ss_v[:, :, 1, None].broadcast_to([64, GP, HD]),
    )

    p_sb = persist.tile([128, KC, C], BF16)

    def emit_p(j):  # accumulate head-pair j into all four P row-blocks
        for t in range(KC):
            nc.tensor.matmul(
                p_ps[t],
                wv2_sb[:, j, t * 128:(t + 1) * 128],
                m2_sb[:, j, :],
                start=(j == 0),
                stop=(j == GP - 1),
            )
            if j == GP - 1:
                # copy each P row-block the moment its accumulation closes,
                # so the first y matmuls start before the last block lands
                if t % 2 == 0:
                    nc.scalar.copy(p_sb[:, t, :], p_ps[t])
                else:
                    nc.vector.tensor_copy(p_sb[:, t, :], p_ps[t])

    # M/P software pipeline: P(pair j-1) runs while m2[j] is being filled
    for j in range(GP):
        m2_ps = psum.tile([128, C], F32, tag="work_ps")
        for two in range(2):
            nc.tensor.matmul(
                m2_ps,
                attn_pad[:, j, two, :],
                wp_sb[:, 2 * j + two, :],
                start=(two == 0),
                stop=(two == 1),
            )
        if j % 2 == 0:
            nc.scalar.copy(m2_sb[:, j, :], m2_ps)
        else:
            nc.vector.tensor_copy(m2_sb[:, j, :], m2_ps)
        if j > 0:
            emit_p(j - 1)
    emit_p(GP - 1)

    # ---- y = x @ P + b ----------------------------------------------------
    # paired-tile stores: one DMA per two n-tiles, round-robin on all queues.
    y_paired = y[:].rearrange("(t p) c -> p t c", p=128)
    ST_ENG = [nc.sync, nc.scalar, nc.gpsimd]
    ybuf = None
    for t in range(NT):
        y_ps = psum.tile([128, C], F32, tag="work_ps")
        for kc in range(KC):
            nc.tensor.matmul(
                y_ps,
                xt_sb[:, kc, t * 128:(t + 1) * 128],
                p_sb[:, kc, :],
                start=(kc == 0),
                stop=(kc == KC - 1),
            )
        if t >= NT - 2:
            # last two tiles: split bias-add + store into column halves on
            # different queues so the final transfer is quarter-size and the
            # first half is already in flight while the second is added
            ybuf = ypool.tile([128, 1, C], BF16)
            HC = C // 2
            for hcol in range(2):
                nc.vector.tensor_add(
                    ybuf[:, 0, hcol * HC:(hcol + 1) * HC],
                    y_ps[:, hcol * HC:(hcol + 1) * HC],
                    bias_sb[:, hcol * HC:(hcol + 1) * HC],
                )
                ST_ENG[(t + hcol) % 3].dma_start(
                    out=y_paired[:, t:t + 1, hcol * HC:(hcol + 1) * HC],
                    in_=ybuf[:, 0:1, hcol * HC:(hcol + 1) * HC],
                )
        elif t >= NT - 4:
            # single-tile stores near the end so the drain finishes sooner
            ybuf = ypool.tile([128, 1, C], BF16)
            nc.vector.tensor_add(ybuf[:, 0, :], y_ps, bias_sb)
            ST_ENG[t % 3].dma_start(out=y_paired[:, t:t + 1, :], in_=ybuf)
        else:
            if t % 2 == 0:
                ybuf = ypool.tile([128, 2, C], BF16)
            nc.vector.tensor_add(ybuf[:, t % 2, :], y_ps, bias_sb)
            if t % 2 == 1:
                eng = ST_ENG[(t // 2) % 3]
                eng.dma_start(out=y_paired[:, t - 1:t + 1, :], in_=ybuf)


def build_nc():
    nc = bacc.Bacc("TRN2", target_bir_lowering=False, debug=False, num_devices=B)
    io = {}
    io["x_nat"] = nc.dram_tensor("x_nat", [128, NT, C], FP8, kind="ExternalInput")
    io["x_tr"] = nc.dram_tensor("x_tr", [C, N], BF16, kind="ExternalInput")
    io["wqk8"] = nc.dram_tensor("wqk8", [128, KC, 2 * C], FP8, kind="ExternalInput")
    io["ident"] = nc.dram_tensor("ident", [128, 128], FP8, kind="ExternalInput")
    io["wqkb"] = nc.dram_tensor("wqkb", [128, KC, 2 * C], BF16, kind="ExternalInput")
    io["wv2"] = nc.dram_tensor("wv2", [128, GP, C], BF16, kind="ExternalInput")
    io["wp2"] = nc.dram_tensor("wp2", [64, NH, C], BF16, kind="ExternalInput")
    io["bpr"] = nc.dram_tensor("bpr", [C], F32, kind="ExternalInput")
    io["temp"] = nc.dram_tensor("temp", [NH], F32, kind="ExternalInput")
    io["y"] = nc.dram_tensor("y", [N, C], BF16, kind="ExternalOutput")
    with tile.TileContext(nc) as tc:
        with ExitStack() as ctx:
            _build_kernel_body(ctx, tc, io)
    nc.compile()
    return nc


_NC_CACHE = None


def _get_nc():
    global _NC_CACHE
    if _NC_CACHE is None:
        _NC_CACHE = build_nc()
    return _NC_CACHE


def prep_host_inputs(x, W_qkv, temperature, W_proj, b_proj):
    """Host-side preprocessing shared by all cores. Returns per-core in_maps."""
    x = np.asarray(x, dtype=np.float32)
    W_qkv = np.asarray(W_qkv, dtype=np.float32)
    temperature = np.asarray(temperature, dtype=np.float32).reshape(NH)
    W_proj = np.asarray(W_proj, dtype=np.float32)
    b_proj = np.asarray(b_proj, dtype=np.float32)

    Wq = W_qkv[:, 0:C].reshape(C, NH, HD)
    Wk = W_qkv[:, C:2 * C].reshape(C, NH, HD)
    wqk_perm = np.concatenate([Wq, Wk], axis=2).reshape(C, 2 * C)  # [(ci),(h)(qk c)]
    wqk_tiled = np.ascontiguousarray(
        wqk_perm.reshape(KC, 128, 2 * C).transpose(1, 0, 2)
    )  # [p, kc, 2C]
    Wv = W_qkv[:, 2 * C:3 * C]  # [ci, (h d)]
    # [(two,d), pair, ci] so head-pairs stack on the partition dim for P
    wv2 = np.ascontiguousarray(
        Wv.T.reshape(GP, 2, HD, C).transpose(1, 2, 0, 3).reshape(128, GP, C)
    )
    wp2 = np.ascontiguousarray(
        W_proj.reshape(NH, HD, C).transpose(1, 0, 2)
    )  # [c, h, e]

    wqk8 = wqk_tiled.astype(FP8_NP)
    wqkb = wqk_tiled.astype(BF16_NP)
    ident = np.eye(128, dtype=np.float32).astype(FP8_NP)
    wv2_bf = wv2.astype(BF16_NP)
    wp2_bf = wp2.astype(BF16_NP)

    in_maps = []
    for b in range(B):
        xb = x[b]
        in_maps.append({
            "x_nat": np.ascontiguousarray(
                (xb * 0.125).reshape(NT, 128, C).transpose(1, 0, 2)
            ).astype(FP8_NP),
            "x_tr": np.ascontiguousarray(xb.T).astype(BF16_NP),
            "wqk8": wqk8,
            "ident": ident,
            "wqkb": wqkb,
            "wv2": wv2_bf,
            "wp2": wp2_bf,
            "bpr": b_proj,
            "temp": temperature,
        })
    return in_maps


def kernel(**inputs):
    x = inputs["x"]
    in_maps = prep_host_inputs(
        x, inputs["W_qkv"], inputs["temperature"], inputs["W_proj"], inputs["b_proj"]
    )
    nc = _get_nc()
    res = run_bass_kernel_spmd(nc, in_maps, list(range(B)))
    y = np.stack([np.asarray(res.results[i]["y"]) for i in range(B)], axis=0)
    return y.astype(np.float32)


if __name__ == "__main__":
    # smoke test with random data
    rng = np.random.default_rng(0)
    ins = {
        "x": rng.standard_normal((B, N, C), dtype=np.float32),
        "x_out": rng.standard_normal((B, N, C), dtype=np.float32),
        "W_qkv": (rng.standard_normal((C, 3 * C), dtype=np.float32) / np.sqrt(C)),
        "temperature": np.ones((NH, 1, 1), np.float32),
        "W_proj": (rng.standard_normal((C, C), dtype=np.float32) / np.sqrt(C)),
        "b_proj": rng.standard_normal((C,), dtype=np.float32) * 0.01,
        "H": 64,
        "W": 64,
    }
    out = kernel(**ins)
    print("out", out.shape, out.dtype, float(np.abs(out).max()))
